# revision 19
# baseline (speedup 1.0000x reference)
"""Trainium2 Bass kernel for GatedMultiScaleRetentionLayer.

Sharding: 8 cores = data-parallel over batch (B=2) x tensor-parallel over
heads (16 heads -> 4 groups of 4). Each core computes its batch's tokens for
its 4 heads end-to-end (QKV+gate projections on a 256-column weight slice,
xpos-rotary, decay-masked retention, per-head GroupNorm, silu gate, partial
output projection). Host sums the 4 partial outputs per batch and adds bo.

Retention uses a chunked formulation. With the decay folded per token
    qhat_i = rot(q_i) * gamma^i * rownorm_i ,  khat_j = rot(k_j) * gamma^-j
the full masked score matrix is causal(qhat khat^T). For a 512-token query
chunk c, the contribution of past chunks c' < c factors through a per-chunk
state matrix M_{c'} = sum_{j in c'} khat_j^T v_j  [dk, dv], applied as one
matmul rhs=qhat_chunk -- this replaces the O(S^2) off-diagonal score blocks.
Only the 512x512 block-diagonal keeps explicit scores (with a triangular
mask on the 128x128 diagonal sub-blocks). khat^T for the M builds comes from
the DMA transpose XBAR, costing no compute-engine time.

All matmuls run in bf16 on the PE with fp32 PSUM accumulation. Instruction
emission is software-pipelined so the PE queue never waits on ACT/DVE/Pool
copies (keeps the tensor engine's DVFS p-state at full clock). PSUM->SBUF
traffic is greedily balanced across ACT, DVE and Pool.
"""

import os

import numpy as np
import ml_dtypes

import concourse.bass as bass
import concourse.tile as tile
from concourse import bacc, mybir
from concourse.bass_utils import run_bass_kernel_spmd

BF16 = ml_dtypes.bfloat16

# ---- problem constants (hardcoded per contract) ----
B = 2
S = 2048
D = 1024
H = 16
DH = 64
ROT = 32
THETA = 10000.0
XPOS_BASE = 512.0
GN_EPS = 1e-5

N_CORES = 8
HG = 4          # head groups (tensor-parallel)
HPC = 4         # heads per core
NCH = HPC * DH  # 256 channels per core
P = 128
QCH = 512       # q chunk (one PSUM bank of fp32)
NQC = S // QCH  # 4 q chunks
KBLK = 128      # k block
NKB = S // KBLK # 16 k blocks
KPC = QCH // KBLK  # 4 k blocks per chunk
NELEM = float(S * DH)  # groupnorm element count per head

LAST_EXEC_NS = None
LAST_RESULTS = None

_PERM = np.concatenate([np.arange(0, ROT, 2), np.arange(1, ROT, 2),
                        np.arange(ROT, DH)])  # within-head column permutation


def _perm_cols(w_slice):
    """Permute rotary dims of each head's 64-column block (even idx first)."""
    out = np.empty_like(w_slice)
    for hl in range(HPC):
        blk = w_slice[..., hl * DH:(hl + 1) * DH]
        out[..., hl * DH:(hl + 1) * DH] = blk[..., _PERM]
    return out


def _rot_tables():
    """angle[t, r], xpos scale[t, r] for pair index r in [0,16)."""
    t = np.arange(S, dtype=np.float64)
    r = np.arange(ROT // 2, dtype=np.float64)
    inv_freq = 1.0 / (THETA ** ((2.0 * r) / ROT))
    angle = t[:, None] * inv_freq[None, :]                   # [S, 16]
    base = (2.0 * r + 0.4 * ROT) / (1.4 * ROT)               # [16]
    power = (t - S // 2) / XPOS_BASE                         # [S]
    scale = base[None, :] ** power[:, None]                  # [S, 16]
    return angle, scale


def _decay_factors():
    """gamma^i*rownorm (for q) and gamma^-j (for k), per global head. f64."""
    h = np.arange(H, dtype=np.float64)
    gamma = 1.0 - 2.0 ** (-5.0 - h)                          # [H]
    t = np.arange(S, dtype=np.float64)
    logg = np.log(gamma)
    g_pos = np.exp(t[None, :] * logg[:, None])               # [H, S] gamma^t
    g_neg = np.exp(-t[None, :] * logg[:, None])              # [H, S] gamma^-t
    rowsum = (1.0 - gamma[:, None] * g_pos) / (1.0 - gamma[:, None])
    rownorm = 1.0 / np.sqrt(rowsum)                          # [H, S]
    return g_pos * rownorm, g_neg


def _cs_tiles(h0):
    """cq, sq, ck, sk tiles [2, 128, S] bf16 for heads h0..h0+3."""
    angle, scale = _rot_tables()
    dq_all, dk_all = _decay_factors()
    cos, sin = np.cos(angle), np.sin(angle)                  # [S, 16]
    cq = np.zeros((2, P, S), np.float64)
    sq = np.zeros((2, P, S), np.float64)
    ck = np.zeros((2, P, S), np.float64)
    sk = np.zeros((2, P, S), np.float64)
    for mt in range(2):
        for half in range(2):
            hl = 2 * mt + half
            g = h0 + hl
            dq = dq_all[g]                                   # [S]
            dk = dk_all[g]
            base = 64 * half
            for rr in range(16):
                cq[mt, base + rr] = cos[:, rr] * scale[:, rr] * dq
                cq[mt, base + 16 + rr] = cos[:, rr] * scale[:, rr] * dq
                sq[mt, base + rr] = sin[:, rr] * scale[:, rr] * dq
                sq[mt, base + 16 + rr] = sin[:, rr] * scale[:, rr] * dq
                ck[mt, base + rr] = cos[:, rr] / scale[:, rr] * dk
                ck[mt, base + 16 + rr] = cos[:, rr] / scale[:, rr] * dk
                sk[mt, base + rr] = sin[:, rr] / scale[:, rr] * dk
                sk[mt, base + 16 + rr] = sin[:, rr] / scale[:, rr] * dk
            cq[mt, base + 32:base + 64] = dq[None, :]
            ck[mt, base + 32:base + 64] = dk[None, :]
    return (cq.astype(BF16), sq.astype(BF16), ck.astype(BF16), sk.astype(BF16))


def _pt_matrix():
    """lhsT of the rotate-half block-swap matrix (out = P @ rhs)."""
    Pm = np.zeros((P, P), np.float32)
    for b0 in (0, 64):
        for rr in range(16):
            Pm[b0 + rr, b0 + 16 + rr] = -1.0
            Pm[b0 + 16 + rr, b0 + rr] = 1.0
    return Pm.T.astype(BF16)  # Pt[k, m] = P[m, k]


def _tri_mask():
    tri = (np.arange(P)[None, :] >= np.arange(P)[:, None])
    return tri.astype(BF16)  # tri[rj, t] = t >= rj


def _blockones():
    k = np.arange(P)
    return (k[:, None] // 64 == k[None, :] // 64).astype(BF16)


def _rep2(vec_slice):
    """[256] channel vector -> [128, 2] f32 (per-partition, per m-tile)."""
    out = np.empty((P, 2), np.float32)
    for mt in range(2):
        out[:, mt] = vec_slice[mt * P:(mt + 1) * P]
    return out


def _host_prep(inputs):
    x = np.asarray(inputs["x"], np.float32)
    Wq = np.asarray(inputs["Wq"], np.float32)
    Wk = np.asarray(inputs["Wk"], np.float32)
    Wv = np.asarray(inputs["Wv"], np.float32)
    Wg = np.asarray(inputs["Wg"], np.float32)
    Wo = np.asarray(inputs["Wo"], np.float32)
    bq = np.asarray(inputs["bq"], np.float32)
    bk = np.asarray(inputs["bk"], np.float32)
    bv = np.asarray(inputs["bv"], np.float32)
    bg = np.asarray(inputs["bg"], np.float32)
    gn_w = np.asarray(inputs["gn_w"], np.float32)
    gn_b = np.asarray(inputs["gn_b"], np.float32)

    pt = _pt_matrix()
    tri = _tri_mask()
    ob = _blockones()
    has_bv = bool(np.any(bv != 0.0))

    in_maps = []
    for core in range(N_CORES):
        b = core // HG
        hg = core % HG
        h0 = HPC * hg
        cols = slice(NCH * hg, NCH * (hg + 1))
        cq, sq, ck, sk = _cs_tiles(h0)
        gnw_rep = np.empty((P, 2), np.float32)
        gnb_rep = np.empty((P, 2), np.float32)
        for mt in range(2):
            for half in range(2):
                g = h0 + 2 * mt + half
                gnw_rep[64 * half:64 * (half + 1), mt] = gn_w[g]
                gnb_rep[64 * half:64 * (half + 1), mt] = gn_b[g]
        idm = np.eye(P, dtype=np.float32).astype(BF16)
        m = {
            "idm": idm,
            "xt": np.ascontiguousarray(x[b].T).astype(BF16),
            "wq": _perm_cols(Wq[:, cols]).astype(BF16),
            "wk": _perm_cols(Wk[:, cols]).astype(BF16),
            "wv": np.ascontiguousarray(Wv[:, cols]).astype(BF16),
            "wg": np.ascontiguousarray(Wg[:, cols]).astype(BF16),
            "wo": np.ascontiguousarray(Wo[cols, :]).astype(BF16),
            "cq": cq, "sq": sq, "ck": ck, "sk": sk,
            "pt": pt, "tri": tri, "ob": ob,
            "gnw": gnw_rep, "gnb": gnb_rep,
            "bqr": _rep2(_perm_cols(bq[None, cols])[0]),
            "bkr": _rep2(_perm_cols(bk[None, cols])[0]),
            "bgr": _rep2(bg[cols]),
        }
        if has_bv:
            m["bvb"] = np.broadcast_to(bv[cols][None, :], (P, NCH)).astype(
                np.float32).copy()
        in_maps.append(m)
    return in_maps, has_bv


class _Bal3:
    """Greedy ACT/DVE/Pool load balancer for elementwise/copy work."""

    def __init__(self, nc):
        self.nc = nc
        self.act = 0.0
        self.dve = 0.0
        self.pool = 0.0

    # ---- cost models (ns), engine-busy portion only ----
    @staticmethod
    def _c_act(n):
        return (352.0 + n) / 1.2

    @staticmethod
    def _c_dve(n):
        return (120.0 + n) / 0.96

    @staticmethod
    def _c_pool(n, eff):
        return n / (1.2 * eff) + 131.0

    def note_act(self, n, extra=0.0):
        self.act += self._c_act(n) + extra

    def note_dve(self, n):
        self.dve += self._c_dve(n)

    def note_pool(self, n, eff=0.6):
        self.pool += self._c_pool(n, eff)

    def ts(self, dst, src, sc_a, sc_b, op0, op1):
        """tensor_scalar (per-partition scalars) on DVE or Pool."""
        n = src.free_size()
        cd = self.dve + self._c_dve(n)
        cp = (self.pool + self._c_pool(n, 0.42)
              if self._pool_ok(dst, src) else float("inf"))
        if cd <= cp:
            self.dve = cd
            self.nc.vector.tensor_scalar(dst, src, sc_a, sc_b, op0, op1)
        else:
            self.pool = cp
            self.nc.gpsimd.tensor_scalar(dst, src, sc_a, sc_b, op0, op1)

    @staticmethod
    def _pool_ok(*aps):
        # GPSIMD cannot access PSUM (walrus verifier)
        return all(ap.space != bass.MemorySpace.PSUM for ap in aps)

    def copy(self, dst, src):
        n = src.free_size()
        ca = self.act + self._c_act(n)
        cd = self.dve + self._c_dve(n)
        cp = (self.pool + self._c_pool(n, 0.6)
              if self._pool_ok(dst, src) else float("inf"))
        m = min(ca, cd, cp)
        if m == ca:
            self.act = ca
            self.nc.scalar.copy(dst, src)
        elif m == cd:
            self.dve = cd
            self.nc.vector.tensor_copy(dst, src)
        else:
            self.pool = cp
            self.nc.gpsimd.tensor_copy(dst, src)

    def tt(self, dst, a, b, op):
        """tensor_tensor on DVE or Pool."""
        n = max(a.free_size(), b.free_size())
        cd = self.dve + self._c_dve(n)
        cp = (self.pool + self._c_pool(n, 0.42)
              if self._pool_ok(dst, a, b) else float("inf"))
        if cd <= cp:
            self.dve = cd
            self.nc.vector.tensor_tensor(dst, a, b, op)
        else:
            self.pool = cp
            self.nc.gpsimd.tensor_tensor(dst, a, b, op)


def _build_program(has_bv):
    nc = bacc.Bacc("TRN2", target_bir_lowering=False, debug=False,
                   num_devices=N_CORES)
    f32 = mybir.dt.float32
    bf16 = mybir.dt.bfloat16

    xt_d = nc.dram_tensor("xt", [D, S], bf16, kind="ExternalInput")
    idm_d = nc.dram_tensor("idm", [P, P], bf16, kind="ExternalInput")
    wq_d = nc.dram_tensor("wq", [D, NCH], bf16, kind="ExternalInput")
    wk_d = nc.dram_tensor("wk", [D, NCH], bf16, kind="ExternalInput")
    wv_d = nc.dram_tensor("wv", [D, NCH], bf16, kind="ExternalInput")
    wg_d = nc.dram_tensor("wg", [D, NCH], bf16, kind="ExternalInput")
    wo_d = nc.dram_tensor("wo", [NCH, D], bf16, kind="ExternalInput")
    cq_d = nc.dram_tensor("cq", [2, P, S], bf16, kind="ExternalInput")
    sq_d = nc.dram_tensor("sq", [2, P, S], bf16, kind="ExternalInput")
    ck_d = nc.dram_tensor("ck", [2, P, S], bf16, kind="ExternalInput")
    sk_d = nc.dram_tensor("sk", [2, P, S], bf16, kind="ExternalInput")
    pt_d = nc.dram_tensor("pt", [P, P], bf16, kind="ExternalInput")
    tri_d = nc.dram_tensor("tri", [P, P], bf16, kind="ExternalInput")
    ob_d = nc.dram_tensor("ob", [P, P], bf16, kind="ExternalInput")
    gnw_d = nc.dram_tensor("gnw", [P, 2], f32, kind="ExternalInput")
    gnb_d = nc.dram_tensor("gnb", [P, 2], f32, kind="ExternalInput")
    bqr_d = nc.dram_tensor("bqr", [P, 2], f32, kind="ExternalInput")
    bkr_d = nc.dram_tensor("bkr", [P, 2], f32, kind="ExternalInput")
    bgr_d = nc.dram_tensor("bgr", [P, 2], f32, kind="ExternalInput")
    bvb_d = (nc.dram_tensor("bvb", [P, NCH], f32, kind="ExternalInput")
             if has_bv else None)
    out_d = nc.dram_tensor("out", [S, D], bf16, kind="ExternalOutput")

    ident = mybir.ActivationFunctionType.Identity
    silu = mybir.ActivationFunctionType.Silu
    fcopy = mybir.ActivationFunctionType.Copy
    fsquare = mybir.ActivationFunctionType.Square
    fsqrt = mybir.ActivationFunctionType.Sqrt
    mul_op = mybir.AluOpType.mult
    add_op = mybir.AluOpType.add
    sub_op = mybir.AluOpType.subtract

    with tile.TileContext(nc) as tc:
        with (
            tc.tile_pool(name="consts", bufs=1) as cpool,
            tc.tile_pool(name="wts", bufs=1) as wpool,
            tc.tile_pool(name="big", bufs=1) as big,
            tc.tile_pool(name="ps", bufs=4, space="PSUM") as ps,
            tc.tile_pool(name="psacc", bufs=2, space="PSUM") as psacc,
            tc.tile_pool(name="pssm", bufs=1, space="PSUM") as pssm,
            tc.tile_pool(name="scp", bufs=6) as scp,
            tc.tile_pool(name="rotp", bufs=10) as rotp,
            tc.tile_pool(name="nrmp", bufs=2) as nrmp,
            tc.tile_pool(name="outp", bufs=4) as outp,
            tc.tile_pool(name="finp", bufs=1) as finp,
        ):
            # ---- high-priority loads: x (8-way parallel) + wv, full BW ----
            # x lives in 8 per-chunk tiles so compute on early chunks can
            # start while later chunks are still in flight.
            xts = []
            for kc in range(8):
                t = big.tile([P, S], bf16, tag=f"xt{kc}", name=f"xt{kc}")
                nc.sync.dma_start(t[:, :], xt_d[kc * P:(kc + 1) * P, :])
                xts.append(t)
            wT = {}
            for nm, dh in (("wv", wv_d), ("wk", wk_d), ("wq", wq_d),
                           ("wg", wg_d)):
                t = wpool.tile([P, 8, NCH], bf16, tag=nm)
                wT[nm] = t
            nc.sync.dma_start(wT["wv"][:, :, :],
                              wv_d.ap().rearrange("(c p) n -> p c n", p=P))
            csT = {}
            for nm in ("cq", "sq", "ck", "sk"):
                csT[nm] = cpool.tile([P, 2, S], bf16, tag=nm, name=nm)
            woT = wpool.tile([P, 2, D], bf16, tag="wo")
            # Everything else big is gated behind the x load (a 1-element
            # gpsimd write into each destination tile that reads xtT) so the
            # DMA engines' packet round-robin can't starve x of HBM BW.
            ptT = cpool.tile([P, P], bf16, tag="pt")
            nc.sync.dma_start(ptT[:, :], pt_d[:, :])
            idmT = cpool.tile([P, P], bf16, tag="idm")
            nc.sync.dma_start(idmT[:, :], idm_d[:, :])
            triT = cpool.tile([P, P], bf16, tag="tri")
            nc.sync.dma_start(triT[:, :], tri_d[:, :])
            obT = cpool.tile([P, P], bf16, tag="ob")
            nc.sync.dma_start(obT[:, :], ob_d[:, :])
            gnwT = cpool.tile([P, 2], f32, tag="gnw")
            nc.sync.dma_start(gnwT[:, :], gnw_d[:, :])
            gnbT = cpool.tile([P, 2], f32, tag="gnb")
            nc.sync.dma_start(gnbT[:, :], gnb_d[:, :])
            biasT = {}
            for nm, dh in (("bqr", bqr_d), ("bkr", bkr_d), ("bgr", bgr_d)):
                t = cpool.tile([P, 2], f32, tag=nm)
                nc.sync.dma_start(t[:, :], dh[:, :])
                biasT[nm] = t
            # 3 gated loads per hwdge queue ring, so no issue instruction
            # ever blocks its queue waiting for a ring slot
            gated = [
                (wT["wk"], wk_d.ap().rearrange("(c p) n -> p c n", p=P),
                 nc.sync),
                (csT["ck"], ck_d.ap().rearrange("i p s -> p i s"), nc.sync),
                (csT["sk"], sk_d.ap().rearrange("i p s -> p i s"), nc.sync),
                (wT["wq"], wq_d.ap().rearrange("(c p) n -> p c n", p=P),
                 nc.scalar),
                (csT["cq"], cq_d.ap().rearrange("i p s -> p i s"), nc.scalar),
                (csT["sq"], sq_d.ap().rearrange("i p s -> p i s"), nc.scalar),
                (wT["wg"], wg_d.ap().rearrange("(c p) n -> p c n", p=P),
                 nc.sync),
                (woT, wo_d.ap().rearrange("(c p) n -> p c n", p=P),
                 nc.sync),
            ]
            for t, src, eng in gated:
                nc.gpsimd.tensor_copy(t[0:1, 0, 0:1], xts[7][0:1, 0:1])
                eng.dma_start(t[:, :, :], src)
            zeroT = cpool.tile([P, 1], f32, tag="zero")
            nc.vector.memset(zeroT[:, :], 0.0)

            epsT = cpool.tile([P, 1], f32, tag="eps")
            nc.vector.memset(epsT[:, :], GN_EPS)
            if has_bv:
                bvbT = cpool.tile([P, NCH], f32, tag="bvb")
                nc.sync.dma_start(bvbT[:, :], bvb_d[:, :])
            qhT = big.tile([P, 2, S], bf16, tag="qh")
            khT = big.tile([P, 2, S], bf16, tag="kh")
            # v stored as head pairs: [tok, kblk, mt, 128] where cols 0:64 =
            # head 2mt, 64:128 = head 2mt+1 (matches rp partition layout).
            vT = big.tile([P, NKB, 2, P], bf16, tag="v")
            # khat^T per (mt, chunk<3) via DMA transpose, for the M builds.
            # Separate tiles so each M build waits only on its own 4 blocks.
            ktT = {}
            for mt in range(2):
                for cp in range(3):
                    ktT[(mt, cp)] = big.tile([P, KPC, P], bf16,
                                             tag=f"kt{mt}{cp}",
                                             name=f"kt{mt}{cp}")
            # M state matrices per (mt, chunk 0..2), block-diagonal head pair
            mTile = big.tile([P, 2, 3, P], bf16, tag="m")
            nc.gpsimd.memset(mTile[:, :, :, :], 0.0)
            gateT = big.tile([P, 2, S], bf16, tag="gate")
            retT = big.tile([P, 2, S], bf16, tag="ret")
            gtdT = big.tile([P, 2, S], bf16, tag="gtd")
            statT = finp.tile([P, 16], f32, tag="stat")

            bal = _Bal3(nc)

            # ================= Phase A1: v projection =================
            # kc-outer passes: compute starts as soon as x chunk 0 and wv
            # land, accumulating into 6 PSUM banks while later x chunks are
            # still streaming in.
            for g0, g1 in ((0, 6), (6, 12), (12, 16)):
                psts = []
                for tt in range(g0, g1):
                    pool_ = ps if (tt - g0) < 4 else psacc
                    psts.append(pool_.tile([P, QCH], f32,
                                           tag="mm" if (tt - g0) < 4
                                           else "acc",
                                           name=f"vp{tt}"))
                for kc in range(8):
                    for j, tt in enumerate(range(g0, g1)):
                        nc.tensor.matmul(
                            psts[j][:, :NCH],
                            lhsT=xts[kc][:, tt * P:(tt + 1) * P],
                            rhs=wT["wv"][:, kc, :],
                            start=(kc == 0), stop=(kc == 7))
                for j, tt in enumerate(range(g0, g1)):
                    for mt in range(2):
                        dstv = vT[:, tt, mt, :]
                        srcv = psts[j][:, mt * P:(mt + 1) * P]
                        if has_bv:
                            bal.tt(dstv, srcv, bvbT[:, mt * P:(mt + 1) * P],
                                   add_op)
                        else:
                            bal.copy(dstv, srcv)

            # ============ Phase A2/A3: k then q projection + rotary ============
            # Pipelined: the rotary PE matmul for chunk i is emitted after the
            # projection matmuls of chunk i+1, so the PE never waits on the
            # ACT/DVE rotary chain.
            def emit_proj(nm, mt, c):
                pst = ps.tile([P, QCH], f32, tag="mm")
                for kc in range(8):
                    nc.tensor.matmul(
                        pst[:, :],
                        lhsT=wT[nm][:, kc, mt * P:(mt + 1) * P],
                        rhs=xts[kc][:, c * QCH:(c + 1) * QCH],
                        start=(kc == 0), stop=(kc == 7))
                return pst

            def emit_rot_pre(pst, cnm, snm, bnm, mt, c):
                """ACT bias copy + DVE muls; returns (t1, t2)."""
                qc = rotp.tile([P, QCH], bf16, tag="rt")
                nc.scalar.activation(qc[:, :], pst[:, :], ident,
                                     bias=biasT[bnm][:, mt:mt + 1], scale=1.0)
                bal.note_act(QCH)
                csl = slice(c * QCH, (c + 1) * QCH)
                # t2 stays on DVE (latency-critical: feeds the PE rotate);
                # t1 is only needed at the add, so it can go anywhere.
                t1 = rotp.tile([P, QCH], bf16, tag="rt")
                bal.tt(t1[:, :], qc[:, :], csT[cnm][:, mt, csl], mul_op)
                t2 = rotp.tile([P, QCH], bf16, tag="rt")
                nc.vector.tensor_tensor(t2[:, :], qc[:, :],
                                        csT[snm][:, mt, csl], mul_op)
                bal.note_dve(QCH)
                return t1, t2

            def emit_rot_post(t1, t2, dst, mt, c):
                """PE rotate matmul + identity-add, then one ACT spill."""
                csl = slice(c * QCH, (c + 1) * QCH)
                psw = ps.tile([P, QCH], f32, tag="mm")
                nc.tensor.matmul(psw[:, :], lhsT=ptT[:, :], rhs=t2[:, :],
                                 start=True, stop=False)
                nc.tensor.matmul(psw[:, :], lhsT=idmT[:, :], rhs=t1[:, :],
                                 start=False, stop=True)
                nc.scalar.copy(dst[:, mt, csl], psw[:, :])
                bal.note_act(QCH)

            # ---- k projection (8 chunks, pipelined) ----
            def emit_ktrans(mt, c):
                """khat^T for chunk (mt, c<3) via DMA transpose XBAR."""
                if c >= 3:
                    return
                for o in range(KPC):
                    K = c * KPC + o
                    nc.sync.dma_start(
                        ktT[(mt, c)][:, o, :],
                        khT[:, mt, K * KBLK:(K + 1) * KBLK],
                        transpose=True)

            kchunks = [(mt, c) for mt in range(2) for c in range(NQC)]
            pend = []  # [(t1, t2, mt, c), ...] with 2-chunk lag
            for i, (mt, c) in enumerate(kchunks):
                pst = emit_proj("wk", mt, c)
                if len(pend) >= 3:
                    p = pend.pop(0)
                    emit_rot_post(p[0], p[1], khT, p[2], p[3])
                    emit_ktrans(p[2], p[3])
                t1, t2 = emit_rot_pre(pst, "ck", "sk", "bkr", mt, c)
                pend.append((t1, t2, mt, c))
            for p in pend:
                emit_rot_post(p[0], p[1], khT, p[2], p[3])
                emit_ktrans(p[2], p[3])

            # ---- q projection (8 chunks, pipelined, M builds interleaved) ----
            # M build jobs: per (mt, chunk c'<3): accumulate 4 k-blocks per
            # head half into a [128,128] PSUM tile, then copy the two diagonal
            # 64x64 blocks into mTile.
            mjobs = [(0, 0), (0, 1), (0, 2), (1, 0), (1, 1), (1, 2)]

            def emit_mbuild(mt, cp):
                mps = pssm.tile([P, P], f32, tag="mps")
                for o in range(KPC):
                    K = cp * KPC + o
                    nc.tensor.matmul(
                        mps[:, :],
                        lhsT=ktT[(mt, cp)][:, o, :],
                        rhs=vT[:, K, mt, :],
                        start=(o == 0), stop=(o == KPC - 1))
                for h in range(2):
                    sl = slice(64 * h, 64 * (h + 1))
                    bal.copy(mTile[sl, mt, cp, sl], mps[sl, sl])

            qchunks = [(mt, c) for mt in range(2) for c in range(NQC)]
            pend = []
            for i, (mt, c) in enumerate(qchunks):
                pst = emit_proj("wq", mt, c)
                if len(pend) >= 3:
                    p = pend.pop(0)
                    emit_rot_post(p[0], p[1], qhT, p[2], p[3])
                t1, t2 = emit_rot_pre(pst, "cq", "sq", "bqr", mt, c)
                pend.append((t1, t2, mt, c))
            for p in pend:
                emit_rot_post(p[0], p[1], qhT, p[2], p[3])

            # ================= Phase B: retention =================
            # Per (mt, c): rp accumulates [128 (head pair dims), 512 q].
            #   - c applies: rp += M[c'] (full 128 rows)
            #   - 8 diagonal-strip blocks (2 halves x 4 offsets) with scores
            # Scores run 3 ahead of the consuming V matmuls, and the previous
            # chunk's stats are emitted mid-chunk, so neither the PE nor the
            # ACT queue ever stalls at a chunk boundary.
            def emit_stats(rp_prev, mt, c):
                qsl = slice(c * QCH, (c + 1) * QCH)
                sidx = mt * NQC + c
                nc.scalar.activation(retT[:, mt, qsl], rp_prev[:, :], fcopy,
                                     accum_out=statT[:, sidx:sidx + 1])
                bal.note_act(QCH, extra=279.0)
                sqs = scp.tile([P, QCH], bf16, tag="sc")
                nc.gpsimd.tensor_tensor(sqs[:, :], retT[:, mt, qsl],
                                        retT[:, mt, qsl], mul_op)
                bal.note_pool(QCH, eff=0.42)
                nc.vector.tensor_reduce(
                    statT[:, 8 + sidx:9 + sidx], sqs[:, :],
                    axis=mybir.AxisListType.X, op=add_op)
                bal.note_dve(QCH)

            pend_stats = None
            for mt in range(2):
                for c in range(NQC):
                    qsl = slice(c * QCH, (c + 1) * QCH)
                    rp = psacc.tile([P, QCH], f32, tag="acc")
                    # diagonal strip: items (h, o), scores 3 ahead of V;
                    # the first two scores are emitted before the applies so
                    # their copies overlap the apply matmuls.
                    items = [(h, o) for o in range(KPC) for h in range(2)]
                    sstiles = {}

                    def emit_applies():
                        for cp in range(c):
                            nc.tensor.matmul(
                                rp[:, :], lhsT=mTile[:, mt, cp, :],
                                rhs=qhT[:, mt, qsl],
                                start=(cp == 0), stop=False,
                                skip_group_check=True)

                    def emit_score(h, o):
                        K = KPC * c + o
                        w = QCH - KBLK * o
                        prow = slice(64 * h, 64 * (h + 1))
                        sp = ps.tile([P, QCH], f32, tag="mm")
                        nc.tensor.matmul(
                            sp[:, :w], lhsT=khT[prow, mt,
                                               K * KBLK:(K + 1) * KBLK],
                            rhs=qhT[prow, mt,
                                    c * QCH + KBLK * o:(c + 1) * QCH],
                            start=True, stop=True)
                        ss = scp.tile([P, QCH], bf16, tag="sc")
                        bal.tt(ss[:, :KBLK], sp[:, :KBLK], triT[:, :],
                               mul_op)
                        if o < KPC - 1:
                            bal.copy(ss[:, KBLK:w], sp[:, KBLK:w])
                        return ss

                    def emit_v(h, o, ss):
                        K = KPC * c + o
                        w = QCH - KBLK * o
                        vstart = (c == 0 and o == 0)
                        vstop = (o == KPC - 1)
                        nc.tensor.matmul(
                            rp[64 * h:64 * (h + 1), KBLK * o:QCH],
                            lhsT=vT[:, K, mt, 64 * h:64 * (h + 1)],
                            rhs=ss[:, :w],
                            start=vstart, stop=vstop,
                            skip_group_check=True)

                    bidx = mt * NQC + c
                    for idx, (h, o) in enumerate(items):
                        sstiles[idx] = emit_score(h, o)
                        if idx == 1:
                            emit_applies()
                        if idx == 2 and pend_stats is not None:
                            emit_stats(*pend_stats)
                            pend_stats = None
                        if idx == 3 and bidx < len(mjobs):
                            emit_mbuild(*mjobs[bidx])
                        if idx >= 3:
                            emit_v(*items[idx - 3], sstiles.pop(idx - 3))
                    for idx in (len(items) - 3, len(items) - 2,
                                len(items) - 1):
                        emit_v(*items[idx], sstiles.pop(idx))
                    pend_stats = (rp, mt, c)
            emit_stats(*pend_stats)

            # ==== Phase C: gate projection with GN finalize tucked inside ====
            # The GN reduction chain (including the tiny per-head-totals PE
            # matmul) is emitted after the second gate chunk so it completes
            # while the remaining gate chunks keep the PE busy; nrm/gtd run
            # q-chunk-major so the output projection never waits.
            gchunks = [(mt, c) for c in range(NQC) for mt in range(2)]

            def emit_gate(mt, c):
                pst = emit_proj("wg", mt, c)
                nc.scalar.activation(
                    gateT[:, mt, c * QCH:(c + 1) * QCH],
                    pst[:, :], silu,
                    bias=biasT["bgr"][:, mt:mt + 1], scale=1.0)
                bal.note_act(QCH)

            def emit_nrm_gtd(c):
                for mt in range(2):
                    csl = slice(c * QCH, (c + 1) * QCH)
                    nrm = nrmp.tile([P, QCH], bf16, tag="nrm")
                    bal.ts(nrm[:, :], retT[:, mt, csl],
                           aff_a[:, mt:mt + 1], aff_b[:, mt:mt + 1],
                           mul_op, add_op)
                    bal.tt(gtdT[:, mt, csl], nrm[:, :], gateT[:, mt, csl],
                           mul_op)

            def emit_outproj(c):
                # bf16 partials; host sums across cores and adds bo
                for tt in range(KPC * c, KPC * (c + 1)):
                    for oc in range(2):
                        op_ps = psacc.tile([P, QCH], f32, tag="acc")
                        for kc in range(2):
                            nc.tensor.matmul(
                                op_ps[:, :],
                                lhsT=gtdT[:, kc, tt * P:(tt + 1) * P],
                                rhs=woT[:, kc, oc * QCH:(oc + 1) * QCH],
                                start=(kc == 0), stop=(kc == 1))
                        ob_t = outp.tile([P, QCH], bf16, tag="ob")
                        bal.copy(ob_t[:, :], op_ps[:, :])
                        nc.sync.dma_start(
                            out_d[tt * P:(tt + 1) * P,
                                  oc * QCH:(oc + 1) * QCH],
                            ob_t[:, :])

            emit_gate(*gchunks[0])
            # GN part 1 on DVE (starts as soon as stats land)
            s1 = finp.tile([P, 4], f32, tag="s1")
            nc.vector.tensor_reduce(
                s1[:, :], statT[:, :].rearrange("p (g c) -> p g c", c=NQC),
                axis=mybir.AxisListType.X, op=add_op)
            s1b = finp.tile([P, 4], bf16, tag="s1b")
            nc.vector.tensor_copy(s1b[:, :], s1[:, :])
            emit_gate(*gchunks[1])
            totp = pssm.tile([P, 4], f32, tag="tot")
            nc.tensor.matmul(totp[:, :], lhsT=obT[:, :], rhs=s1b[:, :],
                             start=True, stop=True)
            tot = finp.tile([P, 4], f32, tag="tot_sb")
            nc.vector.tensor_copy(tot[:, :], totp[:, :])
            mean = finp.tile([P, 2], f32, tag="mean")
            nc.vector.tensor_scalar_mul(mean[:, :], tot[:, 0:2], 1.0 / NELEM)
            ex2 = finp.tile([P, 2], f32, tag="ex2")
            nc.vector.tensor_scalar_mul(ex2[:, :], tot[:, 2:4], 1.0 / NELEM)
            msq = finp.tile([P, 2], f32, tag="msq")
            nc.vector.tensor_tensor(msq[:, :], mean[:, :], mean[:, :], mul_op)
            var = finp.tile([P, 2], f32, tag="var")
            nc.vector.tensor_tensor(var[:, :], ex2[:, :], msq[:, :], sub_op)
            std = finp.tile([P, 2], f32, tag="std")
            nc.scalar.activation(std[:, :], var[:, :], fsqrt,
                                 bias=epsT[:, :], scale=1.0)
            istd = finp.tile([P, 2], f32, tag="istd")
            nc.vector.reciprocal(istd[:, :], std[:, :])
            aff_a = finp.tile([P, 2], f32, tag="aff_a")
            nc.vector.tensor_tensor(aff_a[:, :], istd[:, :], gnwT[:, :], mul_op)
            ma = finp.tile([P, 2], f32, tag="ma")
            nc.vector.tensor_tensor(ma[:, :], mean[:, :], aff_a[:, :], mul_op)
            aff_b = finp.tile([P, 2], f32, tag="aff_b")
            nc.vector.tensor_tensor(aff_b[:, :], gnbT[:, :], ma[:, :], sub_op)
            # remaining gate chunks with nrm/gtd + out-proj groups woven in,
            # so the PE rolls from gate matmuls straight into the output
            # projection with no barrier
            emit_gate(*gchunks[2])
            emit_gate(*gchunks[3])
            emit_nrm_gtd(0)
            emit_gate(*gchunks[4])
            emit_gate(*gchunks[5])
            emit_nrm_gtd(1)
            emit_outproj(0)
            emit_gate(*gchunks[6])
            emit_gate(*gchunks[7])
            emit_nrm_gtd(2)
            emit_outproj(1)
            emit_nrm_gtd(3)
            emit_outproj(2)
            emit_outproj(3)

    nc.compile()
    return nc


_PROGRAM_CACHE = {}


def _get_program(has_bv):
    if has_bv not in _PROGRAM_CACHE:
        _PROGRAM_CACHE[has_bv] = _build_program(has_bv)
    return _PROGRAM_CACHE[has_bv]


def kernel(**inputs):
    global LAST_EXEC_NS, LAST_RESULTS
    in_maps, has_bv = _host_prep(inputs)
    nc = _get_program(has_bv)
    trace = bool(int(os.environ.get("KERNEL_TRACE", "0")))
    kw = {}
    if trace:
        kw["trace"] = True
        kw["trace_cores"] = [int(c) for c in
                             os.environ.get("KERNEL_TRACE_CORES", "0").split(",")]
        td = os.environ.get("KERNEL_TRACE_DIR")
        if td:
            os.makedirs(td, exist_ok=True)
            kw["tmpdir"] = td
    res = run_bass_kernel_spmd(nc, in_maps, list(range(N_CORES)), **kw)
    LAST_EXEC_NS = res.exec_time_ns
    LAST_RESULTS = res
    bo = np.asarray(inputs["bo"], np.float32)
    out = np.zeros((B, S, D), np.float32)
    for core in range(N_CORES):
        out[core // HG] += np.asarray(res.results[core]["out"], np.float32)
    out += bo[None, None, :]
    return out


# revision 20
# speedup vs baseline: 1.0918x; 1.0918x over previous
"""Trainium2 Bass kernel for GatedMultiScaleRetentionLayer.

Sharding: 8 cores = data-parallel over batch (B=2) x tensor-parallel over
heads (16 heads -> 4 groups of 4). Each core computes its batch's tokens for
its 4 heads end-to-end (QKV+gate projections on a 256-column weight slice,
xpos-rotary, decay-masked retention, per-head GroupNorm, silu gate, partial
output projection). Host sums the 4 partial outputs per batch and adds bo.

Retention uses a chunked formulation. With the decay folded per token
    qhat_i = rot(q_i) * gamma^i * rownorm_i ,  khat_j = rot(k_j) * gamma^-j
the full masked score matrix is causal(qhat khat^T). For a 512-token query
chunk c, the contribution of past chunks c' < c factors through a per-chunk
state matrix M_{c'} = sum_{j in c'} khat_j^T v_j  [dk, dv], applied as one
matmul rhs=qhat_chunk -- this replaces the O(S^2) off-diagonal score blocks.
Only the 512x512 block-diagonal keeps explicit scores (with a triangular
mask on the 128x128 diagonal sub-blocks). khat^T for the M builds comes from
the DMA transpose XBAR, costing no compute-engine time.

All matmuls run in bf16 on the PE with fp32 PSUM accumulation. Instruction
emission is software-pipelined so the PE queue never waits on ACT/DVE/Pool
copies (keeps the tensor engine's DVFS p-state at full clock). PSUM->SBUF
traffic is greedily balanced across ACT, DVE and Pool.
"""

import os

import numpy as np
import ml_dtypes

import concourse.bass as bass
import concourse.tile as tile
from concourse import bacc, mybir
from concourse.bass_utils import run_bass_kernel_spmd

BF16 = ml_dtypes.bfloat16

# ---- problem constants (hardcoded per contract) ----
B = 2
S = 2048
D = 1024
H = 16
DH = 64
ROT = 32
THETA = 10000.0
XPOS_BASE = 512.0
GN_EPS = 1e-5

N_CORES = 8
HG = 4          # head groups (tensor-parallel)
HPC = 4         # heads per core
NCH = HPC * DH  # 256 channels per core
P = 128
QCH = 512       # q chunk (one PSUM bank of fp32)
NQC = S // QCH  # 4 q chunks
KBLK = 128      # k block
NKB = S // KBLK # 16 k blocks
KPC = QCH // KBLK  # 4 k blocks per chunk
NELEM = float(S * DH)  # groupnorm element count per head

LAST_EXEC_NS = None
LAST_RESULTS = None

_PERM = np.concatenate([np.arange(0, ROT, 2), np.arange(1, ROT, 2),
                        np.arange(ROT, DH)])  # within-head column permutation


def _perm_cols(w_slice):
    """Permute rotary dims of each head's 64-column block (even idx first)."""
    out = np.empty_like(w_slice)
    for hl in range(HPC):
        blk = w_slice[..., hl * DH:(hl + 1) * DH]
        out[..., hl * DH:(hl + 1) * DH] = blk[..., _PERM]
    return out


def _rot_tables():
    """angle[t, r], xpos scale[t, r] for pair index r in [0,16)."""
    t = np.arange(S, dtype=np.float64)
    r = np.arange(ROT // 2, dtype=np.float64)
    inv_freq = 1.0 / (THETA ** ((2.0 * r) / ROT))
    angle = t[:, None] * inv_freq[None, :]                   # [S, 16]
    base = (2.0 * r + 0.4 * ROT) / (1.4 * ROT)               # [16]
    power = (t - S // 2) / XPOS_BASE                         # [S]
    scale = base[None, :] ** power[:, None]                  # [S, 16]
    return angle, scale


def _decay_factors():
    """gamma^i*rownorm (for q) and gamma^-j (for k), per global head. f64."""
    h = np.arange(H, dtype=np.float64)
    gamma = 1.0 - 2.0 ** (-5.0 - h)                          # [H]
    t = np.arange(S, dtype=np.float64)
    logg = np.log(gamma)
    g_pos = np.exp(t[None, :] * logg[:, None])               # [H, S] gamma^t
    g_neg = np.exp(-t[None, :] * logg[:, None])              # [H, S] gamma^-t
    rowsum = (1.0 - gamma[:, None] * g_pos) / (1.0 - gamma[:, None])
    rownorm = 1.0 / np.sqrt(rowsum)                          # [H, S]
    return g_pos * rownorm, g_neg


def _cs_tiles(h0):
    """cq, sq, ck, sk tiles [2, 128, S] bf16 for heads h0..h0+3."""
    angle, scale = _rot_tables()
    dq_all, dk_all = _decay_factors()
    cos, sin = np.cos(angle), np.sin(angle)                  # [S, 16]
    cq = np.zeros((2, P, S), np.float64)
    sq = np.zeros((2, P, S), np.float64)
    ck = np.zeros((2, P, S), np.float64)
    sk = np.zeros((2, P, S), np.float64)
    for mt in range(2):
        for half in range(2):
            hl = 2 * mt + half
            g = h0 + hl
            dq = dq_all[g]                                   # [S]
            dk = dk_all[g]
            base = 64 * half
            for rr in range(16):
                cq[mt, base + rr] = cos[:, rr] * scale[:, rr] * dq
                cq[mt, base + 16 + rr] = cos[:, rr] * scale[:, rr] * dq
                sq[mt, base + rr] = sin[:, rr] * scale[:, rr] * dq
                sq[mt, base + 16 + rr] = sin[:, rr] * scale[:, rr] * dq
                ck[mt, base + rr] = cos[:, rr] / scale[:, rr] * dk
                ck[mt, base + 16 + rr] = cos[:, rr] / scale[:, rr] * dk
                sk[mt, base + rr] = sin[:, rr] / scale[:, rr] * dk
                sk[mt, base + 16 + rr] = sin[:, rr] / scale[:, rr] * dk
            cq[mt, base + 32:base + 64] = dq[None, :]
            ck[mt, base + 32:base + 64] = dk[None, :]
    return (cq.astype(BF16), sq.astype(BF16), ck.astype(BF16), sk.astype(BF16))


def _pt_matrix():
    """lhsT of the rotate-half block-swap matrix (out = P @ rhs)."""
    Pm = np.zeros((P, P), np.float32)
    for b0 in (0, 64):
        for rr in range(16):
            Pm[b0 + rr, b0 + 16 + rr] = -1.0
            Pm[b0 + 16 + rr, b0 + rr] = 1.0
    return Pm.T.astype(BF16)  # Pt[k, m] = P[m, k]


def _tri_mask():
    tri = (np.arange(P)[None, :] >= np.arange(P)[:, None])
    return tri.astype(BF16)  # tri[rj, t] = t >= rj


def _blockones():
    k = np.arange(P)
    return (k[:, None] // 64 == k[None, :] // 64).astype(BF16)


def _rep2(vec_slice):
    """[256] channel vector -> [128, 2] f32 (per-partition, per m-tile)."""
    out = np.empty((P, 2), np.float32)
    for mt in range(2):
        out[:, mt] = vec_slice[mt * P:(mt + 1) * P]
    return out


def _host_prep(inputs):
    x = np.asarray(inputs["x"], np.float32)
    Wq = np.asarray(inputs["Wq"], np.float32)
    Wk = np.asarray(inputs["Wk"], np.float32)
    Wv = np.asarray(inputs["Wv"], np.float32)
    Wg = np.asarray(inputs["Wg"], np.float32)
    Wo = np.asarray(inputs["Wo"], np.float32)
    bq = np.asarray(inputs["bq"], np.float32)
    bk = np.asarray(inputs["bk"], np.float32)
    bv = np.asarray(inputs["bv"], np.float32)
    bg = np.asarray(inputs["bg"], np.float32)
    gn_w = np.asarray(inputs["gn_w"], np.float32)
    gn_b = np.asarray(inputs["gn_b"], np.float32)

    pt = _pt_matrix()
    tri = _tri_mask()
    ob = _blockones()
    has_bv = bool(np.any(bv != 0.0))

    in_maps = []
    for core in range(N_CORES):
        b = core // HG
        hg = core % HG
        h0 = HPC * hg
        cols = slice(NCH * hg, NCH * (hg + 1))
        cq, sq, ck, sk = _cs_tiles(h0)
        gnw_rep = np.empty((P, 2), np.float32)
        gnb_rep = np.empty((P, 2), np.float32)
        for mt in range(2):
            for half in range(2):
                g = h0 + 2 * mt + half
                gnw_rep[64 * half:64 * (half + 1), mt] = gn_w[g]
                gnb_rep[64 * half:64 * (half + 1), mt] = gn_b[g]
        idm = np.eye(P, dtype=np.float32).astype(BF16)
        m = {
            "idm": idm,
            "xt": np.ascontiguousarray(x[b].T).astype(BF16),
            "wq": _perm_cols(Wq[:, cols]).astype(BF16),
            "wk": _perm_cols(Wk[:, cols]).astype(BF16),
            "wv": np.ascontiguousarray(Wv[:, cols]).astype(BF16),
            "wg": np.ascontiguousarray(Wg[:, cols]).astype(BF16),
            "wo": np.ascontiguousarray(Wo[cols, :]).astype(BF16),
            "cq": cq, "sq": sq, "ck": ck, "sk": sk,
            "pt": pt, "tri": tri, "ob": ob,
            "gnw": gnw_rep, "gnb": gnb_rep,
            "bqr": _rep2(_perm_cols(bq[None, cols])[0]),
            "bkr": _rep2(_perm_cols(bk[None, cols])[0]),
            "bgr": _rep2(bg[cols]),
        }
        if has_bv:
            m["bvb"] = np.broadcast_to(bv[cols][None, :], (P, NCH)).astype(
                np.float32).copy()
        in_maps.append(m)
    return in_maps, has_bv


class _Bal3:
    """Greedy ACT/DVE/Pool load balancer for elementwise/copy work."""

    def __init__(self, nc):
        self.nc = nc
        self.act = 0.0
        self.dve = 0.0
        self.pool = 0.0

    # ---- cost models (ns), engine-busy portion only ----
    @staticmethod
    def _c_act(n):
        return (352.0 + n) / 1.2

    @staticmethod
    def _c_dve(n):
        return (120.0 + n) / 0.96

    @staticmethod
    def _c_pool(n, eff):
        return n / (1.2 * eff) + 131.0

    def note_act(self, n, extra=0.0):
        self.act += self._c_act(n) + extra

    def note_dve(self, n):
        self.dve += self._c_dve(n)

    def note_pool(self, n, eff=0.6):
        self.pool += self._c_pool(n, eff)

    def ts(self, dst, src, sc_a, sc_b, op0, op1):
        """tensor_scalar (per-partition scalars) on DVE or Pool."""
        n = src.free_size()
        cd = self.dve + self._c_dve(n)
        cp = (self.pool + self._c_pool(n, 0.42)
              if self._pool_ok(dst, src) else float("inf"))
        if cd <= cp:
            self.dve = cd
            self.nc.vector.tensor_scalar(dst, src, sc_a, sc_b, op0, op1)
        else:
            self.pool = cp
            self.nc.gpsimd.tensor_scalar(dst, src, sc_a, sc_b, op0, op1)

    @staticmethod
    def _pool_ok(*aps):
        # GPSIMD cannot access PSUM (walrus verifier)
        return all(ap.space != bass.MemorySpace.PSUM for ap in aps)

    def copy(self, dst, src):
        n = src.free_size()
        ca = self.act + self._c_act(n)
        cd = self.dve + self._c_dve(n)
        cp = (self.pool + self._c_pool(n, 0.6)
              if self._pool_ok(dst, src) else float("inf"))
        m = min(ca, cd, cp)
        if m == ca:
            self.act = ca
            self.nc.scalar.copy(dst, src)
        elif m == cd:
            self.dve = cd
            self.nc.vector.tensor_copy(dst, src)
        else:
            self.pool = cp
            self.nc.gpsimd.tensor_copy(dst, src)

    def tt(self, dst, a, b, op):
        """tensor_tensor on DVE or Pool."""
        n = max(a.free_size(), b.free_size())
        cd = self.dve + self._c_dve(n)
        cp = (self.pool + self._c_pool(n, 0.42)
              if self._pool_ok(dst, a, b) else float("inf"))
        if cd <= cp:
            self.dve = cd
            self.nc.vector.tensor_tensor(dst, a, b, op)
        else:
            self.pool = cp
            self.nc.gpsimd.tensor_tensor(dst, a, b, op)


def _build_program(has_bv):
    nc = bacc.Bacc("TRN2", target_bir_lowering=False, debug=False,
                   num_devices=N_CORES)
    f32 = mybir.dt.float32
    bf16 = mybir.dt.bfloat16

    xt_d = nc.dram_tensor("xt", [D, S], bf16, kind="ExternalInput")
    idm_d = nc.dram_tensor("idm", [P, P], bf16, kind="ExternalInput")
    wq_d = nc.dram_tensor("wq", [D, NCH], bf16, kind="ExternalInput")
    wk_d = nc.dram_tensor("wk", [D, NCH], bf16, kind="ExternalInput")
    wv_d = nc.dram_tensor("wv", [D, NCH], bf16, kind="ExternalInput")
    wg_d = nc.dram_tensor("wg", [D, NCH], bf16, kind="ExternalInput")
    wo_d = nc.dram_tensor("wo", [NCH, D], bf16, kind="ExternalInput")
    cq_d = nc.dram_tensor("cq", [2, P, S], bf16, kind="ExternalInput")
    sq_d = nc.dram_tensor("sq", [2, P, S], bf16, kind="ExternalInput")
    ck_d = nc.dram_tensor("ck", [2, P, S], bf16, kind="ExternalInput")
    sk_d = nc.dram_tensor("sk", [2, P, S], bf16, kind="ExternalInput")
    pt_d = nc.dram_tensor("pt", [P, P], bf16, kind="ExternalInput")
    tri_d = nc.dram_tensor("tri", [P, P], bf16, kind="ExternalInput")
    ob_d = nc.dram_tensor("ob", [P, P], bf16, kind="ExternalInput")
    gnw_d = nc.dram_tensor("gnw", [P, 2], f32, kind="ExternalInput")
    gnb_d = nc.dram_tensor("gnb", [P, 2], f32, kind="ExternalInput")
    bqr_d = nc.dram_tensor("bqr", [P, 2], f32, kind="ExternalInput")
    bkr_d = nc.dram_tensor("bkr", [P, 2], f32, kind="ExternalInput")
    bgr_d = nc.dram_tensor("bgr", [P, 2], f32, kind="ExternalInput")
    bvb_d = (nc.dram_tensor("bvb", [P, NCH], f32, kind="ExternalInput")
             if has_bv else None)
    out_d = nc.dram_tensor("out", [S, D], bf16, kind="ExternalOutput")

    ident = mybir.ActivationFunctionType.Identity
    silu = mybir.ActivationFunctionType.Silu
    fcopy = mybir.ActivationFunctionType.Copy
    fsquare = mybir.ActivationFunctionType.Square
    fsqrt = mybir.ActivationFunctionType.Sqrt
    mul_op = mybir.AluOpType.mult
    add_op = mybir.AluOpType.add
    sub_op = mybir.AluOpType.subtract

    with tile.TileContext(nc) as tc:
        with (
            tc.tile_pool(name="consts", bufs=1) as cpool,
            tc.tile_pool(name="wts", bufs=1) as wpool,
            tc.tile_pool(name="big", bufs=1) as big,
            tc.tile_pool(name="ps", bufs=4, space="PSUM") as ps,
            tc.tile_pool(name="psacc", bufs=2, space="PSUM") as psacc,
            tc.tile_pool(name="pssm", bufs=1, space="PSUM") as pssm,
            tc.tile_pool(name="scp", bufs=6) as scp,
            tc.tile_pool(name="rotp", bufs=10) as rotp,
            tc.tile_pool(name="nrmp", bufs=2) as nrmp,
            tc.tile_pool(name="outp", bufs=4) as outp,
            tc.tile_pool(name="finp", bufs=1) as finp,
        ):
            # ---- high-priority loads: x (8-way parallel) + wv, full BW ----
            # x lives in 8 per-chunk tiles so compute on early chunks can
            # start while later chunks are still in flight.
            xts = []
            for kc in range(8):
                t = big.tile([P, S], bf16, tag=f"xt{kc}", name=f"xt{kc}")
                nc.sync.dma_start(t[:, :], xt_d[kc * P:(kc + 1) * P, :])
                xts.append(t)
            wT = {}
            for nm, dh in (("wv", wv_d), ("wk", wk_d), ("wq", wq_d),
                           ("wg", wg_d)):
                t = wpool.tile([P, 8, NCH], bf16, tag=nm)
                wT[nm] = t
            nc.sync.dma_start(wT["wv"][:, :, :],
                              wv_d.ap().rearrange("(c p) n -> p c n", p=P))
            csT = {}
            for nm in ("cq", "sq", "ck", "sk"):
                csT[nm] = cpool.tile([P, 2, S], bf16, tag=nm, name=nm)
            woT = wpool.tile([P, 2, D], bf16, tag="wo")
            # Everything else big is gated behind the x load (a 1-element
            # gpsimd write into each destination tile that reads xtT) so the
            # DMA engines' packet round-robin can't starve x of HBM BW.
            ptT = cpool.tile([P, P], bf16, tag="pt")
            nc.sync.dma_start(ptT[:, :], pt_d[:, :])
            idmT = cpool.tile([P, P], bf16, tag="idm")
            nc.sync.dma_start(idmT[:, :], idm_d[:, :])
            triT = cpool.tile([P, P], bf16, tag="tri")
            nc.sync.dma_start(triT[:, :], tri_d[:, :])
            obT = cpool.tile([P, P], bf16, tag="ob")
            nc.sync.dma_start(obT[:, :], ob_d[:, :])
            gnwT = cpool.tile([P, 2], f32, tag="gnw")
            nc.sync.dma_start(gnwT[:, :], gnw_d[:, :])
            gnbT = cpool.tile([P, 2], f32, tag="gnb")
            nc.sync.dma_start(gnbT[:, :], gnb_d[:, :])
            biasT = {}
            for nm, dh in (("bqr", bqr_d), ("bkr", bkr_d), ("bgr", bgr_d)):
                t = cpool.tile([P, 2], f32, tag=nm)
                nc.sync.dma_start(t[:, :], dh[:, :])
                biasT[nm] = t
            # 3 gated loads per hwdge queue ring, so no issue instruction
            # ever blocks its queue waiting for a ring slot
            gated = [
                (wT["wk"], wk_d.ap().rearrange("(c p) n -> p c n", p=P),
                 nc.sync),
                (csT["ck"], ck_d.ap().rearrange("i p s -> p i s"), nc.sync),
                (csT["sk"], sk_d.ap().rearrange("i p s -> p i s"), nc.sync),
                (wT["wq"], wq_d.ap().rearrange("(c p) n -> p c n", p=P),
                 nc.scalar),
                (csT["cq"], cq_d.ap().rearrange("i p s -> p i s"), nc.scalar),
                (csT["sq"], sq_d.ap().rearrange("i p s -> p i s"), nc.scalar),
                (wT["wg"], wg_d.ap().rearrange("(c p) n -> p c n", p=P),
                 nc.sync),
                (woT, wo_d.ap().rearrange("(c p) n -> p c n", p=P),
                 nc.sync),
            ]
            for t, src, eng in gated:
                nc.gpsimd.tensor_copy(t[0:1, 0, 0:1], xts[7][0:1, 0:1])
                eng.dma_start(t[:, :, :], src)
            zeroT = cpool.tile([P, 1], f32, tag="zero")
            nc.vector.memset(zeroT[:, :], 0.0)

            epsT = cpool.tile([P, 1], f32, tag="eps")
            nc.vector.memset(epsT[:, :], GN_EPS)
            if has_bv:
                bvbT = cpool.tile([P, NCH], f32, tag="bvb")
                nc.sync.dma_start(bvbT[:, :], bvb_d[:, :])
            qhT = big.tile([P, 2, S], bf16, tag="qh")
            khT = big.tile([P, 2, S], bf16, tag="kh")
            # v stored as head pairs: [tok, kblk, mt, 128] where cols 0:64 =
            # head 2mt, 64:128 = head 2mt+1 (matches rp partition layout).
            vT = big.tile([P, NKB, 2, P], bf16, tag="v")
            # khat^T per (mt, chunk<3) via DMA transpose, for the M builds.
            # Separate tiles so each M build waits only on its own 4 blocks.
            ktT = {}
            for mt in range(2):
                for cp in range(3):
                    ktT[(mt, cp)] = big.tile([P, KPC, P], bf16,
                                             tag=f"kt{mt}{cp}",
                                             name=f"kt{mt}{cp}")
            # M state matrices per (mt, chunk 0..2), block-diagonal head pair
            mTile = big.tile([P, 2, 3, P], bf16, tag="m")
            nc.gpsimd.memset(mTile[:, :, :, :], 0.0)
            gateT = big.tile([P, 2, S], bf16, tag="gate")
            retT = big.tile([P, 2, S], bf16, tag="ret")
            gtdT = big.tile([P, 2, S], bf16, tag="gtd")
            statT = finp.tile([P, 16], f32, tag="stat")

            bal = _Bal3(nc)

            # ================= Phase A1: v projection =================
            for tt in range(NKB):
                pst = ps.tile([P, QCH], f32, tag="mm")
                for kc in range(8):
                    nc.tensor.matmul(
                        pst[:, :NCH],
                        lhsT=xts[kc][:, tt * P:(tt + 1) * P],
                        rhs=wT["wv"][:, kc, :],
                        start=(kc == 0), stop=(kc == 7))
                for mt in range(2):
                    dstv = vT[:, tt, mt, :]
                    srcv = pst[:, mt * P:(mt + 1) * P]
                    if has_bv:
                        bal.tt(dstv, srcv, bvbT[:, mt * P:(mt + 1) * P],
                               add_op)
                    else:
                        bal.copy(dstv, srcv)

            # ============ Phase A2/A3: k then q projection + rotary ============
            # Pipelined: the rotary PE matmul for chunk i is emitted after the
            # projection matmuls of chunk i+1, so the PE never waits on the
            # ACT/DVE rotary chain.
            def emit_proj(nm, mt, c):
                pst = ps.tile([P, QCH], f32, tag="mm")
                for kc in range(8):
                    nc.tensor.matmul(
                        pst[:, :],
                        lhsT=wT[nm][:, kc, mt * P:(mt + 1) * P],
                        rhs=xts[kc][:, c * QCH:(c + 1) * QCH],
                        start=(kc == 0), stop=(kc == 7))
                return pst

            def emit_rot_pre(pst, cnm, snm, bnm, mt, c):
                """ACT bias copy + DVE muls; returns (t1, t2)."""
                qc = rotp.tile([P, QCH], bf16, tag="rt")
                nc.scalar.activation(qc[:, :], pst[:, :], ident,
                                     bias=biasT[bnm][:, mt:mt + 1], scale=1.0)
                bal.note_act(QCH)
                csl = slice(c * QCH, (c + 1) * QCH)
                # t2 on DVE (latency-critical: feeds the PE rotate);
                # t1 on the otherwise-idle Pool engine -- in phase A the
                # ACT+DVE demand per chunk otherwise exceeds the PE period.
                t1 = rotp.tile([P, QCH], bf16, tag="rt")
                nc.gpsimd.tensor_tensor(t1[:, :], qc[:, :],
                                        csT[cnm][:, mt, csl], mul_op)
                bal.note_pool(QCH, eff=0.42)
                t2 = rotp.tile([P, QCH], bf16, tag="rt")
                nc.vector.tensor_tensor(t2[:, :], qc[:, :],
                                        csT[snm][:, mt, csl], mul_op)
                bal.note_dve(QCH)
                return t1, t2

            def emit_rot_post(t1, t2, dst, mt, c):
                """PE rotate matmul + identity-add, then one ACT spill."""
                csl = slice(c * QCH, (c + 1) * QCH)
                psw = ps.tile([P, QCH], f32, tag="mm")
                nc.tensor.matmul(psw[:, :], lhsT=ptT[:, :], rhs=t2[:, :],
                                 start=True, stop=False)
                nc.tensor.matmul(psw[:, :], lhsT=idmT[:, :], rhs=t1[:, :],
                                 start=False, stop=True)
                bal.copy(dst[:, mt, csl], psw[:, :])

            # ---- k projection (8 chunks, pipelined) ----
            def emit_ktrans(mt, c):
                """khat^T for chunk (mt, c<3) via DMA transpose XBAR."""
                if c >= 3:
                    return
                for o in range(KPC):
                    K = c * KPC + o
                    nc.sync.dma_start(
                        ktT[(mt, c)][:, o, :],
                        khT[:, mt, K * KBLK:(K + 1) * KBLK],
                        transpose=True)

            kchunks = [(mt, c) for mt in range(2) for c in range(NQC)]
            pend = []  # [(t1, t2, mt, c), ...] with 2-chunk lag
            for i, (mt, c) in enumerate(kchunks):
                pst = emit_proj("wk", mt, c)
                if len(pend) >= 3:
                    p = pend.pop(0)
                    emit_rot_post(p[0], p[1], khT, p[2], p[3])
                    emit_ktrans(p[2], p[3])
                t1, t2 = emit_rot_pre(pst, "ck", "sk", "bkr", mt, c)
                pend.append((t1, t2, mt, c))
            for p in pend:
                emit_rot_post(p[0], p[1], khT, p[2], p[3])
                emit_ktrans(p[2], p[3])

            # ---- q projection (8 chunks, pipelined, M builds interleaved) ----
            # M build jobs: per (mt, chunk c'<3): accumulate 4 k-blocks per
            # head half into a [128,128] PSUM tile, then copy the two diagonal
            # 64x64 blocks into mTile.
            mjobs = [(0, 0), (0, 1), (0, 2), (1, 0), (1, 1), (1, 2)]

            def emit_mbuild(mt, cp):
                mps = pssm.tile([P, P], f32, tag="mps")
                for o in range(KPC):
                    K = cp * KPC + o
                    nc.tensor.matmul(
                        mps[:, :],
                        lhsT=ktT[(mt, cp)][:, o, :],
                        rhs=vT[:, K, mt, :],
                        start=(o == 0), stop=(o == KPC - 1))
                for h in range(2):
                    sl = slice(64 * h, 64 * (h + 1))
                    bal.copy(mTile[sl, mt, cp, sl], mps[sl, sl])

            qchunks = [(mt, c) for mt in range(2) for c in range(NQC)]
            pend = []
            for i, (mt, c) in enumerate(qchunks):
                pst = emit_proj("wq", mt, c)
                if len(pend) >= 3:
                    p = pend.pop(0)
                    emit_rot_post(p[0], p[1], qhT, p[2], p[3])
                t1, t2 = emit_rot_pre(pst, "cq", "sq", "bqr", mt, c)
                pend.append((t1, t2, mt, c))
            for p in pend:
                emit_rot_post(p[0], p[1], qhT, p[2], p[3])

            # ================= Phase B: retention =================
            # Per (mt, c): rp accumulates [128 (head pair dims), 512 q].
            #   - c applies: rp += M[c'] (full 128 rows)
            #   - 8 diagonal-strip blocks (2 halves x 4 offsets) with scores
            # Scores run 3 ahead of the consuming V matmuls, and the previous
            # chunk's stats are emitted mid-chunk, so neither the PE nor the
            # ACT queue ever stalls at a chunk boundary.
            def emit_stats(rp_prev, mt, c):
                qsl = slice(c * QCH, (c + 1) * QCH)
                sidx = mt * NQC + c
                nc.scalar.activation(retT[:, mt, qsl], rp_prev[:, :], fcopy,
                                     accum_out=statT[:, sidx:sidx + 1])
                bal.note_act(QCH, extra=279.0)
                sqs = scp.tile([P, QCH], bf16, tag="sc")
                nc.gpsimd.tensor_tensor(sqs[:, :], retT[:, mt, qsl],
                                        retT[:, mt, qsl], mul_op)
                bal.note_pool(QCH, eff=0.42)
                nc.vector.tensor_reduce(
                    statT[:, 8 + sidx:9 + sidx], sqs[:, :],
                    axis=mybir.AxisListType.X, op=add_op)
                bal.note_dve(QCH)

            pend_stats = None
            for mt in range(2):
                for c in range(NQC):
                    qsl = slice(c * QCH, (c + 1) * QCH)
                    rp = psacc.tile([P, QCH], f32, tag="acc")
                    # diagonal strip: items (h, o), scores 3 ahead of V;
                    # the first two scores are emitted before the applies so
                    # their copies overlap the apply matmuls.
                    items = [(h, o) for o in range(KPC) for h in range(2)]
                    sstiles = {}

                    def emit_applies():
                        for cp in range(c):
                            nc.tensor.matmul(
                                rp[:, :], lhsT=mTile[:, mt, cp, :],
                                rhs=qhT[:, mt, qsl],
                                start=(cp == 0), stop=False,
                                skip_group_check=True)

                    def emit_score(h, o):
                        K = KPC * c + o
                        w = QCH - KBLK * o
                        prow = slice(64 * h, 64 * (h + 1))
                        sp = ps.tile([P, QCH], f32, tag="mm")
                        nc.tensor.matmul(
                            sp[:, :w], lhsT=khT[prow, mt,
                                               K * KBLK:(K + 1) * KBLK],
                            rhs=qhT[prow, mt,
                                    c * QCH + KBLK * o:(c + 1) * QCH],
                            start=True, stop=True)
                        ss = scp.tile([P, QCH], bf16, tag="sc")
                        bal.tt(ss[:, :KBLK], sp[:, :KBLK], triT[:, :],
                               mul_op)
                        if o < KPC - 1:
                            bal.copy(ss[:, KBLK:w], sp[:, KBLK:w])
                        return ss

                    def emit_v(h, o, ss):
                        K = KPC * c + o
                        w = QCH - KBLK * o
                        vstart = (c == 0 and o == 0)
                        vstop = (o == KPC - 1)
                        nc.tensor.matmul(
                            rp[64 * h:64 * (h + 1), KBLK * o:QCH],
                            lhsT=vT[:, K, mt, 64 * h:64 * (h + 1)],
                            rhs=ss[:, :w],
                            start=vstart, stop=vstop,
                            skip_group_check=True)

                    bidx = mt * NQC + c
                    for idx, (h, o) in enumerate(items):
                        sstiles[idx] = emit_score(h, o)
                        if idx == 1:
                            emit_applies()
                        if idx == 3 and bidx < len(mjobs):
                            emit_mbuild(*mjobs[bidx])
                        if idx >= 3:
                            emit_v(*items[idx - 3], sstiles.pop(idx - 3))
                    for idx in (len(items) - 3, len(items) - 2,
                                len(items) - 1):
                        emit_v(*items[idx], sstiles.pop(idx))
                    # previous chunk's stats go here: the spill waits on
                    # nothing by now and blocks no copies this chunk needed
                    if pend_stats is not None:
                        emit_stats(*pend_stats)
                    pend_stats = (rp, mt, c)
            emit_stats(*pend_stats)

            # ==== Phase C: gate projection with GN finalize tucked inside ====
            # The GN reduction chain (including the tiny per-head-totals PE
            # matmul) is emitted after the second gate chunk so it completes
            # while the remaining gate chunks keep the PE busy; nrm/gtd run
            # q-chunk-major so the output projection never waits.
            gchunks = [(mt, c) for c in range(NQC) for mt in range(2)]

            def emit_gate(mt, c):
                pst = emit_proj("wg", mt, c)
                nc.scalar.activation(
                    gateT[:, mt, c * QCH:(c + 1) * QCH],
                    pst[:, :], silu,
                    bias=biasT["bgr"][:, mt:mt + 1], scale=1.0)
                bal.note_act(QCH)

            def emit_nrm_gtd(c):
                for mt in range(2):
                    csl = slice(c * QCH, (c + 1) * QCH)
                    nrm = nrmp.tile([P, QCH], bf16, tag="nrm")
                    bal.ts(nrm[:, :], retT[:, mt, csl],
                           aff_a[:, mt:mt + 1], aff_b[:, mt:mt + 1],
                           mul_op, add_op)
                    bal.tt(gtdT[:, mt, csl], nrm[:, :], gateT[:, mt, csl],
                           mul_op)

            def emit_outproj(c):
                # bf16 partials; host sums across cores and adds bo
                for tt in range(KPC * c, KPC * (c + 1)):
                    for oc in range(2):
                        op_ps = psacc.tile([P, QCH], f32, tag="acc")
                        for kc in range(2):
                            nc.tensor.matmul(
                                op_ps[:, :],
                                lhsT=gtdT[:, kc, tt * P:(tt + 1) * P],
                                rhs=woT[:, kc, oc * QCH:(oc + 1) * QCH],
                                start=(kc == 0), stop=(kc == 1))
                        ob_t = outp.tile([P, QCH], bf16, tag="ob")
                        bal.copy(ob_t[:, :], op_ps[:, :])
                        nc.sync.dma_start(
                            out_d[tt * P:(tt + 1) * P,
                                  oc * QCH:(oc + 1) * QCH],
                            ob_t[:, :])

            emit_gate(*gchunks[0])
            # GN part 1 on DVE (starts as soon as stats land)
            s1 = finp.tile([P, 4], f32, tag="s1")
            nc.vector.tensor_reduce(
                s1[:, :], statT[:, :].rearrange("p (g c) -> p g c", c=NQC),
                axis=mybir.AxisListType.X, op=add_op)
            s1b = finp.tile([P, 4], bf16, tag="s1b")
            nc.vector.tensor_copy(s1b[:, :], s1[:, :])
            emit_gate(*gchunks[1])
            totp = pssm.tile([P, 4], f32, tag="tot")
            nc.tensor.matmul(totp[:, :], lhsT=obT[:, :], rhs=s1b[:, :],
                             start=True, stop=True)
            tot = finp.tile([P, 4], f32, tag="tot_sb")
            nc.vector.tensor_copy(tot[:, :], totp[:, :])
            mean = finp.tile([P, 2], f32, tag="mean")
            nc.vector.tensor_scalar_mul(mean[:, :], tot[:, 0:2], 1.0 / NELEM)
            ex2 = finp.tile([P, 2], f32, tag="ex2")
            nc.vector.tensor_scalar_mul(ex2[:, :], tot[:, 2:4], 1.0 / NELEM)
            msq = finp.tile([P, 2], f32, tag="msq")
            nc.vector.tensor_tensor(msq[:, :], mean[:, :], mean[:, :], mul_op)
            var = finp.tile([P, 2], f32, tag="var")
            nc.vector.tensor_tensor(var[:, :], ex2[:, :], msq[:, :], sub_op)
            std = finp.tile([P, 2], f32, tag="std")
            nc.scalar.activation(std[:, :], var[:, :], fsqrt,
                                 bias=epsT[:, :], scale=1.0)
            istd = finp.tile([P, 2], f32, tag="istd")
            nc.vector.reciprocal(istd[:, :], std[:, :])
            aff_a = finp.tile([P, 2], f32, tag="aff_a")
            nc.vector.tensor_tensor(aff_a[:, :], istd[:, :], gnwT[:, :], mul_op)
            ma = finp.tile([P, 2], f32, tag="ma")
            nc.vector.tensor_tensor(ma[:, :], mean[:, :], aff_a[:, :], mul_op)
            aff_b = finp.tile([P, 2], f32, tag="aff_b")
            nc.vector.tensor_tensor(aff_b[:, :], gnbT[:, :], ma[:, :], sub_op)
            # remaining gate chunks with nrm/gtd + out-proj groups woven in,
            # so the PE rolls from gate matmuls straight into the output
            # projection with no barrier
            emit_gate(*gchunks[2])
            emit_gate(*gchunks[3])
            emit_nrm_gtd(0)
            emit_gate(*gchunks[4])
            emit_gate(*gchunks[5])
            emit_nrm_gtd(1)
            emit_outproj(0)
            emit_gate(*gchunks[6])
            emit_gate(*gchunks[7])
            emit_nrm_gtd(2)
            emit_outproj(1)
            emit_nrm_gtd(3)
            emit_outproj(2)
            emit_outproj(3)

    nc.compile()
    return nc


_PROGRAM_CACHE = {}


def _get_program(has_bv):
    if has_bv not in _PROGRAM_CACHE:
        _PROGRAM_CACHE[has_bv] = _build_program(has_bv)
    return _PROGRAM_CACHE[has_bv]


def kernel(**inputs):
    global LAST_EXEC_NS, LAST_RESULTS
    in_maps, has_bv = _host_prep(inputs)
    nc = _get_program(has_bv)
    trace = bool(int(os.environ.get("KERNEL_TRACE", "0")))
    kw = {}
    if trace:
        kw["trace"] = True
        kw["trace_cores"] = [int(c) for c in
                             os.environ.get("KERNEL_TRACE_CORES", "0").split(",")]
        td = os.environ.get("KERNEL_TRACE_DIR")
        if td:
            os.makedirs(td, exist_ok=True)
            kw["tmpdir"] = td
    res = run_bass_kernel_spmd(nc, in_maps, list(range(N_CORES)), **kw)
    LAST_EXEC_NS = res.exec_time_ns
    LAST_RESULTS = res
    bo = np.asarray(inputs["bo"], np.float32)
    out = np.zeros((B, S, D), np.float32)
    for core in range(N_CORES):
        out[core // HG] += np.asarray(res.results[core]["out"], np.float32)
    out += bo[None, None, :]
    return out


# revision 23
# speedup vs baseline: 1.0920x; 1.0002x over previous
"""Trainium2 Bass kernel for GatedMultiScaleRetentionLayer.

Sharding: 8 cores = data-parallel over batch (B=2) x tensor-parallel over
heads (16 heads -> 4 groups of 4). Each core computes its batch's tokens for
its 4 heads end-to-end (QKV+gate projections on a 256-column weight slice,
xpos-rotary, decay-masked retention, per-head GroupNorm, silu gate, partial
output projection). Host sums the 4 partial outputs per batch and adds bo.

Retention uses a chunked formulation. With the decay folded per token
    qhat_i = rot(q_i) * gamma^i * rownorm_i ,  khat_j = rot(k_j) * gamma^-j
the full masked score matrix is causal(qhat khat^T). For a 512-token query
chunk c, the contribution of past chunks c' < c factors through a per-chunk
state matrix M_{c'} = sum_{j in c'} khat_j^T v_j  [dk, dv], applied as one
matmul rhs=qhat_chunk -- this replaces the O(S^2) off-diagonal score blocks.
Only the 512x512 block-diagonal keeps explicit scores (with a triangular
mask on the 128x128 diagonal sub-blocks). khat^T for the M builds comes from
the DMA transpose XBAR, costing no compute-engine time.

All matmuls run in bf16 on the PE with fp32 PSUM accumulation. Instruction
emission is software-pipelined so the PE queue never waits on ACT/DVE/Pool
copies (keeps the tensor engine's DVFS p-state at full clock). PSUM->SBUF
traffic is greedily balanced across ACT, DVE and Pool.
"""

import os

import numpy as np
import ml_dtypes

import concourse.bass as bass
import concourse.tile as tile
from concourse import bacc, mybir
from concourse.bass_utils import run_bass_kernel_spmd

BF16 = ml_dtypes.bfloat16

# ---- problem constants (hardcoded per contract) ----
B = 2
S = 2048
D = 1024
H = 16
DH = 64
ROT = 32
THETA = 10000.0
XPOS_BASE = 512.0
GN_EPS = 1e-5

N_CORES = 8
HG = 4          # head groups (tensor-parallel)
HPC = 4         # heads per core
NCH = HPC * DH  # 256 channels per core
P = 128
QCH = 512       # q chunk (one PSUM bank of fp32)
NQC = S // QCH  # 4 q chunks
KBLK = 128      # k block
NKB = S // KBLK # 16 k blocks
KPC = QCH // KBLK  # 4 k blocks per chunk
NELEM = float(S * DH)  # groupnorm element count per head

LAST_EXEC_NS = None
LAST_RESULTS = None

_PERM = np.concatenate([np.arange(0, ROT, 2), np.arange(1, ROT, 2),
                        np.arange(ROT, DH)])  # within-head column permutation


def _perm_cols(w_slice):
    """Permute rotary dims of each head's 64-column block (even idx first)."""
    out = np.empty_like(w_slice)
    for hl in range(HPC):
        blk = w_slice[..., hl * DH:(hl + 1) * DH]
        out[..., hl * DH:(hl + 1) * DH] = blk[..., _PERM]
    return out


def _rot_tables():
    """angle[t, r], xpos scale[t, r] for pair index r in [0,16)."""
    t = np.arange(S, dtype=np.float64)
    r = np.arange(ROT // 2, dtype=np.float64)
    inv_freq = 1.0 / (THETA ** ((2.0 * r) / ROT))
    angle = t[:, None] * inv_freq[None, :]                   # [S, 16]
    base = (2.0 * r + 0.4 * ROT) / (1.4 * ROT)               # [16]
    power = (t - S // 2) / XPOS_BASE                         # [S]
    scale = base[None, :] ** power[:, None]                  # [S, 16]
    return angle, scale


def _decay_factors():
    """gamma^i*rownorm (for q) and gamma^-j (for k), per global head. f64."""
    h = np.arange(H, dtype=np.float64)
    gamma = 1.0 - 2.0 ** (-5.0 - h)                          # [H]
    t = np.arange(S, dtype=np.float64)
    logg = np.log(gamma)
    g_pos = np.exp(t[None, :] * logg[:, None])               # [H, S] gamma^t
    g_neg = np.exp(-t[None, :] * logg[:, None])              # [H, S] gamma^-t
    rowsum = (1.0 - gamma[:, None] * g_pos) / (1.0 - gamma[:, None])
    rownorm = 1.0 / np.sqrt(rowsum)                          # [H, S]
    return g_pos * rownorm, g_neg


def _cs_tiles(h0):
    """cq, sq, ck, sk tiles [2, 128, S] bf16 for heads h0..h0+3."""
    angle, scale = _rot_tables()
    dq_all, dk_all = _decay_factors()
    cos, sin = np.cos(angle), np.sin(angle)                  # [S, 16]
    cq = np.zeros((2, P, S), np.float64)
    sq = np.zeros((2, P, S), np.float64)
    ck = np.zeros((2, P, S), np.float64)
    sk = np.zeros((2, P, S), np.float64)
    for mt in range(2):
        for half in range(2):
            hl = 2 * mt + half
            g = h0 + hl
            dq = dq_all[g]                                   # [S]
            dk = dk_all[g]
            base = 64 * half
            for rr in range(16):
                cq[mt, base + rr] = cos[:, rr] * scale[:, rr] * dq
                cq[mt, base + 16 + rr] = cos[:, rr] * scale[:, rr] * dq
                sq[mt, base + rr] = sin[:, rr] * scale[:, rr] * dq
                sq[mt, base + 16 + rr] = sin[:, rr] * scale[:, rr] * dq
                ck[mt, base + rr] = cos[:, rr] / scale[:, rr] * dk
                ck[mt, base + 16 + rr] = cos[:, rr] / scale[:, rr] * dk
                sk[mt, base + rr] = sin[:, rr] / scale[:, rr] * dk
                sk[mt, base + 16 + rr] = sin[:, rr] / scale[:, rr] * dk
            cq[mt, base + 32:base + 64] = dq[None, :]
            ck[mt, base + 32:base + 64] = dk[None, :]
    return (cq.astype(BF16), sq.astype(BF16), ck.astype(BF16), sk.astype(BF16))


def _pt_matrix():
    """lhsT of the rotate-half block-swap matrix (out = P @ rhs)."""
    Pm = np.zeros((P, P), np.float32)
    for b0 in (0, 64):
        for rr in range(16):
            Pm[b0 + rr, b0 + 16 + rr] = -1.0
            Pm[b0 + 16 + rr, b0 + rr] = 1.0
    return Pm.T.astype(BF16)  # Pt[k, m] = P[m, k]


def _tri_mask():
    tri = (np.arange(P)[None, :] >= np.arange(P)[:, None])
    return tri.astype(BF16)  # tri[rj, t] = t >= rj


def _blockones():
    k = np.arange(P)
    return (k[:, None] // 64 == k[None, :] // 64).astype(BF16)


def _rep2(vec_slice):
    """[256] channel vector -> [128, 2] f32 (per-partition, per m-tile)."""
    out = np.empty((P, 2), np.float32)
    for mt in range(2):
        out[:, mt] = vec_slice[mt * P:(mt + 1) * P]
    return out


def _host_prep(inputs):
    x = np.asarray(inputs["x"], np.float32)
    Wq = np.asarray(inputs["Wq"], np.float32)
    Wk = np.asarray(inputs["Wk"], np.float32)
    Wv = np.asarray(inputs["Wv"], np.float32)
    Wg = np.asarray(inputs["Wg"], np.float32)
    Wo = np.asarray(inputs["Wo"], np.float32)
    bq = np.asarray(inputs["bq"], np.float32)
    bk = np.asarray(inputs["bk"], np.float32)
    bv = np.asarray(inputs["bv"], np.float32)
    bg = np.asarray(inputs["bg"], np.float32)
    gn_w = np.asarray(inputs["gn_w"], np.float32)
    gn_b = np.asarray(inputs["gn_b"], np.float32)

    pt = _pt_matrix()
    tri = _tri_mask()
    ob = _blockones()
    has_bv = bool(np.any(bv != 0.0))

    in_maps = []
    for core in range(N_CORES):
        b = core // HG
        hg = core % HG
        h0 = HPC * hg
        cols = slice(NCH * hg, NCH * (hg + 1))
        cq, sq, ck, sk = _cs_tiles(h0)
        gnw_rep = np.empty((P, 2), np.float32)
        gnb_rep = np.empty((P, 2), np.float32)
        for mt in range(2):
            for half in range(2):
                g = h0 + 2 * mt + half
                gnw_rep[64 * half:64 * (half + 1), mt] = gn_w[g]
                gnb_rep[64 * half:64 * (half + 1), mt] = gn_b[g]
        idm = np.eye(P, dtype=np.float32).astype(BF16)
        m = {
            "idm": idm,
            "xt": np.ascontiguousarray(x[b].T).astype(BF16),
            "wq": _perm_cols(Wq[:, cols]).astype(BF16),
            "wk": _perm_cols(Wk[:, cols]).astype(BF16),
            "wv": np.ascontiguousarray(Wv[:, cols]).astype(BF16),
            "wg": np.ascontiguousarray(Wg[:, cols]).astype(BF16),
            "wo": np.ascontiguousarray(Wo[cols, :]).astype(BF16),
            "cq": cq, "sq": sq, "ck": ck, "sk": sk,
            "pt": pt, "tri": tri, "ob": ob,
            "gnw": gnw_rep, "gnb": gnb_rep,
            "bqr": _rep2(_perm_cols(bq[None, cols])[0]),
            "bkr": _rep2(_perm_cols(bk[None, cols])[0]),
            "bgr": _rep2(bg[cols]),
        }
        if has_bv:
            m["bvb"] = np.broadcast_to(bv[cols][None, :], (P, NCH)).astype(
                np.float32).copy()
        in_maps.append(m)
    return in_maps, has_bv


class _Bal3:
    """Greedy ACT/DVE/Pool load balancer for elementwise/copy work."""

    def __init__(self, nc):
        self.nc = nc
        self.act = 0.0
        self.dve = 0.0
        self.pool = 0.0

    # ---- cost models (ns), engine-busy portion only ----
    @staticmethod
    def _c_act(n):
        return (352.0 + n) / 1.2

    @staticmethod
    def _c_dve(n):
        return (120.0 + n) / 0.96

    @staticmethod
    def _c_pool(n, eff):
        return n / (1.2 * eff) + 131.0

    def note_act(self, n, extra=0.0):
        self.act += self._c_act(n) + extra

    def note_dve(self, n):
        self.dve += self._c_dve(n)

    def note_pool(self, n, eff=0.6):
        self.pool += self._c_pool(n, eff)

    def ts(self, dst, src, sc_a, sc_b, op0, op1):
        """tensor_scalar (per-partition scalars) on DVE or Pool."""
        n = src.free_size()
        cd = self.dve + self._c_dve(n)
        cp = (self.pool + self._c_pool(n, 0.42)
              if self._pool_ok(dst, src) else float("inf"))
        if cd <= cp:
            self.dve = cd
            self.nc.vector.tensor_scalar(dst, src, sc_a, sc_b, op0, op1)
        else:
            self.pool = cp
            self.nc.gpsimd.tensor_scalar(dst, src, sc_a, sc_b, op0, op1)

    @staticmethod
    def _pool_ok(*aps):
        # GPSIMD cannot access PSUM (walrus verifier)
        return all(ap.space != bass.MemorySpace.PSUM for ap in aps)

    def copy(self, dst, src):
        n = src.free_size()
        ca = self.act + self._c_act(n)
        cd = self.dve + self._c_dve(n)
        cp = (self.pool + self._c_pool(n, 0.6)
              if self._pool_ok(dst, src) else float("inf"))
        m = min(ca, cd, cp)
        if m == ca:
            self.act = ca
            self.nc.scalar.copy(dst, src)
        elif m == cd:
            self.dve = cd
            self.nc.vector.tensor_copy(dst, src)
        else:
            self.pool = cp
            self.nc.gpsimd.tensor_copy(dst, src)

    def tt(self, dst, a, b, op):
        """tensor_tensor on DVE or Pool."""
        n = max(a.free_size(), b.free_size())
        cd = self.dve + self._c_dve(n)
        cp = (self.pool + self._c_pool(n, 0.42)
              if self._pool_ok(dst, a, b) else float("inf"))
        if cd <= cp:
            self.dve = cd
            self.nc.vector.tensor_tensor(dst, a, b, op)
        else:
            self.pool = cp
            self.nc.gpsimd.tensor_tensor(dst, a, b, op)


def _build_program(has_bv):
    nc = bacc.Bacc("TRN2", target_bir_lowering=False, debug=False,
                   num_devices=N_CORES)
    f32 = mybir.dt.float32
    bf16 = mybir.dt.bfloat16

    xt_d = nc.dram_tensor("xt", [D, S], bf16, kind="ExternalInput")
    idm_d = nc.dram_tensor("idm", [P, P], bf16, kind="ExternalInput")
    wq_d = nc.dram_tensor("wq", [D, NCH], bf16, kind="ExternalInput")
    wk_d = nc.dram_tensor("wk", [D, NCH], bf16, kind="ExternalInput")
    wv_d = nc.dram_tensor("wv", [D, NCH], bf16, kind="ExternalInput")
    wg_d = nc.dram_tensor("wg", [D, NCH], bf16, kind="ExternalInput")
    wo_d = nc.dram_tensor("wo", [NCH, D], bf16, kind="ExternalInput")
    cq_d = nc.dram_tensor("cq", [2, P, S], bf16, kind="ExternalInput")
    sq_d = nc.dram_tensor("sq", [2, P, S], bf16, kind="ExternalInput")
    ck_d = nc.dram_tensor("ck", [2, P, S], bf16, kind="ExternalInput")
    sk_d = nc.dram_tensor("sk", [2, P, S], bf16, kind="ExternalInput")
    pt_d = nc.dram_tensor("pt", [P, P], bf16, kind="ExternalInput")
    tri_d = nc.dram_tensor("tri", [P, P], bf16, kind="ExternalInput")
    ob_d = nc.dram_tensor("ob", [P, P], bf16, kind="ExternalInput")
    gnw_d = nc.dram_tensor("gnw", [P, 2], f32, kind="ExternalInput")
    gnb_d = nc.dram_tensor("gnb", [P, 2], f32, kind="ExternalInput")
    bqr_d = nc.dram_tensor("bqr", [P, 2], f32, kind="ExternalInput")
    bkr_d = nc.dram_tensor("bkr", [P, 2], f32, kind="ExternalInput")
    bgr_d = nc.dram_tensor("bgr", [P, 2], f32, kind="ExternalInput")
    bvb_d = (nc.dram_tensor("bvb", [P, NCH], f32, kind="ExternalInput")
             if has_bv else None)
    out_d = nc.dram_tensor("out", [S, D], bf16, kind="ExternalOutput")

    ident = mybir.ActivationFunctionType.Identity
    silu = mybir.ActivationFunctionType.Silu
    fcopy = mybir.ActivationFunctionType.Copy
    fsquare = mybir.ActivationFunctionType.Square
    fsqrt = mybir.ActivationFunctionType.Sqrt
    mul_op = mybir.AluOpType.mult
    add_op = mybir.AluOpType.add
    sub_op = mybir.AluOpType.subtract

    with tile.TileContext(nc) as tc:
        with (
            tc.tile_pool(name="consts", bufs=1) as cpool,
            tc.tile_pool(name="wts", bufs=1) as wpool,
            tc.tile_pool(name="big", bufs=1) as big,
            tc.tile_pool(name="ps", bufs=4, space="PSUM") as ps,
            tc.tile_pool(name="psacc", bufs=2, space="PSUM") as psacc,
            tc.tile_pool(name="pssm", bufs=1, space="PSUM") as pssm,
            tc.tile_pool(name="scp", bufs=6) as scp,
            tc.tile_pool(name="rotp", bufs=10) as rotp,
            tc.tile_pool(name="nrmp", bufs=2) as nrmp,
            tc.tile_pool(name="outp", bufs=4) as outp,
            tc.tile_pool(name="finp", bufs=1) as finp,
        ):
            # ---- high-priority loads: x (8-way parallel) + wv, full BW ----
            # x lives in 8 per-chunk tiles so compute on early chunks can
            # start while later chunks are still in flight.
            xts = []
            for kc in range(8):
                t = big.tile([P, S], bf16, tag=f"xt{kc}", name=f"xt{kc}")
                nc.sync.dma_start(t[:, :], xt_d[kc * P:(kc + 1) * P, :])
                xts.append(t)
            wT = {}
            for nm, dh in (("wv", wv_d), ("wk", wk_d), ("wq", wq_d),
                           ("wg", wg_d)):
                t = wpool.tile([P, 8, NCH], bf16, tag=nm)
                wT[nm] = t
            nc.scalar.dma_start(wT["wv"][:, :, :],
                              wv_d.ap().rearrange("(c p) n -> p c n", p=P))
            csT = {}
            for nm in ("cq", "sq", "ck", "sk"):
                csT[nm] = cpool.tile([P, 2, S], bf16, tag=nm, name=nm)
            woT = wpool.tile([P, 2, D], bf16, tag="wo")
            # Everything else big is gated behind the x load (a 1-element
            # gpsimd write into each destination tile that reads xtT) so the
            # DMA engines' packet round-robin can't starve x of HBM BW.
            ptT = cpool.tile([P, P], bf16, tag="pt")
            nc.scalar.dma_start(ptT[:, :], pt_d[:, :])
            idmT = cpool.tile([P, P], bf16, tag="idm")
            nc.scalar.dma_start(idmT[:, :], idm_d[:, :])
            triT = cpool.tile([P, P], bf16, tag="tri")
            nc.scalar.dma_start(triT[:, :], tri_d[:, :])
            obT = cpool.tile([P, P], bf16, tag="ob")
            nc.scalar.dma_start(obT[:, :], ob_d[:, :])
            gnwT = cpool.tile([P, 2], f32, tag="gnw")
            nc.scalar.dma_start(gnwT[:, :], gnw_d[:, :])
            gnbT = cpool.tile([P, 2], f32, tag="gnb")
            nc.scalar.dma_start(gnbT[:, :], gnb_d[:, :])
            biasT = {}
            for nm, dh in (("bqr", bqr_d), ("bkr", bkr_d), ("bgr", bgr_d)):
                t = cpool.tile([P, 2], f32, tag=nm)
                nc.scalar.dma_start(t[:, :], dh[:, :])
                biasT[nm] = t
            # 3 gated loads per hwdge queue ring, so no issue instruction
            # ever blocks its queue waiting for a ring slot
            gated = [
                (wT["wk"], wk_d.ap().rearrange("(c p) n -> p c n", p=P),
                 nc.sync),
                (csT["ck"], ck_d.ap().rearrange("i p s -> p i s"), nc.sync),
                (csT["sk"], sk_d.ap().rearrange("i p s -> p i s"), nc.sync),
                (wT["wq"], wq_d.ap().rearrange("(c p) n -> p c n", p=P),
                 nc.scalar),
                (csT["cq"], cq_d.ap().rearrange("i p s -> p i s"), nc.scalar),
                (csT["sq"], sq_d.ap().rearrange("i p s -> p i s"), nc.scalar),
                (wT["wg"], wg_d.ap().rearrange("(c p) n -> p c n", p=P),
                 nc.sync),
                (woT, wo_d.ap().rearrange("(c p) n -> p c n", p=P),
                 nc.sync),
            ]
            for t, src, eng in gated:
                nc.gpsimd.tensor_copy(t[0:1, 0, 0:1], xts[7][0:1, 0:1])
                eng.dma_start(t[:, :, :], src)
            zeroT = cpool.tile([P, 1], f32, tag="zero")
            nc.vector.memset(zeroT[:, :], 0.0)

            epsT = cpool.tile([P, 1], f32, tag="eps")
            nc.vector.memset(epsT[:, :], GN_EPS)
            if has_bv:
                bvbT = cpool.tile([P, NCH], f32, tag="bvb")
                nc.sync.dma_start(bvbT[:, :], bvb_d[:, :])
            qhT = big.tile([P, 2, S], bf16, tag="qh")
            khT = big.tile([P, 2, S], bf16, tag="kh")
            # v stored as head pairs: [tok, kblk, mt, 128] where cols 0:64 =
            # head 2mt, 64:128 = head 2mt+1 (matches rp partition layout).
            vT = big.tile([P, NKB, 2, P], bf16, tag="v")
            # khat^T per (mt, chunk<3) via DMA transpose, for the M builds.
            # Separate tiles so each M build waits only on its own 4 blocks.
            ktT = {}
            for mt in range(2):
                for cp in range(4):
                    ktT[(mt, cp)] = big.tile([P, KPC, P], bf16,
                                             tag=f"kt{mt}{cp}",
                                             name=f"kt{mt}{cp}")
            # M state matrices per (mt, chunk 0..2), block-diagonal head pair
            mTile = big.tile([P, 2, 3, P], bf16, tag="m")
            nc.gpsimd.memset(mTile[:, :, :, :], 0.0)
            # chunk-local 128-block state pair (2 slots, reused every chunk)
            m128T = big.tile([P, 2, P], bf16, tag="m128")
            nc.gpsimd.memset(m128T[:, :, :], 0.0)
            # zero matrix: a full-width start=True matmul against it zeroes a
            # whole rp bank (partial-region start flags are unreliable once
            # several writers share one accumulation bank)
            zmT = big.tile([P, P], bf16, tag="zm")
            nc.gpsimd.memset(zmT[:, :], 0.0)
            gateT = big.tile([P, 2, S], bf16, tag="gate")
            retT = big.tile([P, 2, S], bf16, tag="ret")
            gtdT = big.tile([P, 2, S], bf16, tag="gtd")
            statT = finp.tile([P, 16], f32, tag="stat")

            bal = _Bal3(nc)

            # ================= Phase A1: v projection =================
            for tt in range(NKB):
                pst = ps.tile([P, QCH], f32, tag="mm")
                for kc in range(8):
                    nc.tensor.matmul(
                        pst[:, :NCH],
                        lhsT=xts[kc][:, tt * P:(tt + 1) * P],
                        rhs=wT["wv"][:, kc, :],
                        start=(kc == 0), stop=(kc == 7))
                for mt in range(2):
                    dstv = vT[:, tt, mt, :]
                    srcv = pst[:, mt * P:(mt + 1) * P]
                    if has_bv:
                        bal.tt(dstv, srcv, bvbT[:, mt * P:(mt + 1) * P],
                               add_op)
                    else:
                        bal.copy(dstv, srcv)

            # ============ Phase A2/A3: k then q projection + rotary ============
            # Pipelined: the rotary PE matmul for chunk i is emitted after the
            # projection matmuls of chunk i+1, so the PE never waits on the
            # ACT/DVE rotary chain.
            def emit_proj(nm, mt, c):
                pst = ps.tile([P, QCH], f32, tag="mm")
                for kc in range(8):
                    nc.tensor.matmul(
                        pst[:, :],
                        lhsT=wT[nm][:, kc, mt * P:(mt + 1) * P],
                        rhs=xts[kc][:, c * QCH:(c + 1) * QCH],
                        start=(kc == 0), stop=(kc == 7))
                return pst

            def emit_rot_pre(pst, cnm, snm, bnm, mt, c):
                """ACT bias copy + DVE muls; returns (t1, t2)."""
                qc = rotp.tile([P, QCH], bf16, tag="rt")
                nc.scalar.activation(qc[:, :], pst[:, :], ident,
                                     bias=biasT[bnm][:, mt:mt + 1], scale=1.0)
                bal.note_act(QCH)
                csl = slice(c * QCH, (c + 1) * QCH)
                # t2 on DVE (latency-critical: feeds the PE rotate);
                # t1 on the otherwise-idle Pool engine -- in phase A the
                # ACT+DVE demand per chunk otherwise exceeds the PE period.
                t1 = rotp.tile([P, QCH], bf16, tag="rt")
                nc.gpsimd.tensor_tensor(t1[:, :], qc[:, :],
                                        csT[cnm][:, mt, csl], mul_op)
                bal.note_pool(QCH, eff=0.42)
                t2 = rotp.tile([P, QCH], bf16, tag="rt")
                nc.vector.tensor_tensor(t2[:, :], qc[:, :],
                                        csT[snm][:, mt, csl], mul_op)
                bal.note_dve(QCH)
                return t1, t2

            def emit_rot_post(t1, t2, dst, mt, c):
                """PE rotate matmul + identity-add, then one ACT spill."""
                csl = slice(c * QCH, (c + 1) * QCH)
                psw = ps.tile([P, QCH], f32, tag="mm")
                nc.tensor.matmul(psw[:, :], lhsT=ptT[:, :], rhs=t2[:, :],
                                 start=True, stop=False)
                nc.tensor.matmul(psw[:, :], lhsT=idmT[:, :], rhs=t1[:, :],
                                 start=False, stop=True)
                bal.copy(dst[:, mt, csl], psw[:, :])

            # ---- k projection (8 chunks, pipelined) ----
            def emit_ktrans(mt, c):
                """khat^T for chunk (mt, c) via DMA transpose XBAR."""
                nos = range(KPC) if c < 3 else range(2)
                for o in nos:
                    K = c * KPC + o
                    nc.sync.dma_start(
                        ktT[(mt, c)][:, o, :],
                        khT[:, mt, K * KBLK:(K + 1) * KBLK],
                        transpose=True)

            kchunks = [(mt, c) for mt in range(2) for c in range(NQC)]
            pend = []  # [(t1, t2, mt, c), ...] with 2-chunk lag
            for i, (mt, c) in enumerate(kchunks):
                pst = emit_proj("wk", mt, c)
                if len(pend) >= 3:
                    p = pend.pop(0)
                    emit_rot_post(p[0], p[1], khT, p[2], p[3])
                    emit_ktrans(p[2], p[3])
                t1, t2 = emit_rot_pre(pst, "ck", "sk", "bkr", mt, c)
                pend.append((t1, t2, mt, c))
            for p in pend:
                emit_rot_post(p[0], p[1], khT, p[2], p[3])
                emit_ktrans(p[2], p[3])

            # ---- q projection (8 chunks, pipelined, M builds interleaved) ----
            # M build jobs: per (mt, chunk c'<3): accumulate 4 k-blocks per
            # head half into a [128,128] PSUM tile, then copy the two diagonal
            # 64x64 blocks into mTile.
            mjobs = [(0, 0), (0, 1), (0, 2), (1, 0), (1, 1), (1, 2)]

            def emit_mbuild(mt, cp):
                mps = pssm.tile([P, P], f32, tag="mps")
                for o in range(KPC):
                    K = cp * KPC + o
                    nc.tensor.matmul(
                        mps[:, :],
                        lhsT=ktT[(mt, cp)][:, o, :],
                        rhs=vT[:, K, mt, :],
                        start=(o == 0), stop=(o == KPC - 1))
                for h in range(2):
                    sl = slice(64 * h, 64 * (h + 1))
                    bal.copy(mTile[sl, mt, cp, sl], mps[sl, sl])

            qchunks = [(mt, c) for mt in range(2) for c in range(NQC)]
            pend = []
            for i, (mt, c) in enumerate(qchunks):
                pst = emit_proj("wq", mt, c)
                if len(pend) >= 3:
                    p = pend.pop(0)
                    emit_rot_post(p[0], p[1], qhT, p[2], p[3])
                t1, t2 = emit_rot_pre(pst, "cq", "sq", "bqr", mt, c)
                pend.append((t1, t2, mt, c))
            for p in pend:
                emit_rot_post(p[0], p[1], qhT, p[2], p[3])

            # ================= Phase B: retention =================
            # Per (mt, c): rp accumulates [128 (head pair dims), 512 q].
            #   - c applies: rp += M[c'] (full 128 rows)
            #   - 8 diagonal-strip blocks (2 halves x 4 offsets) with scores
            # Scores run 3 ahead of the consuming V matmuls, and the previous
            # chunk's stats are emitted mid-chunk, so neither the PE nor the
            # ACT queue ever stalls at a chunk boundary.
            def emit_stats(rp_prev, mt, c):
                qsl = slice(c * QCH, (c + 1) * QCH)
                sidx = mt * NQC + c
                nc.scalar.activation(retT[:, mt, qsl], rp_prev[:, :], fcopy,
                                     accum_out=statT[:, sidx:sidx + 1])
                bal.note_act(QCH, extra=279.0)
                sqs = scp.tile([P, QCH], bf16, tag="sc")
                nc.gpsimd.tensor_tensor(sqs[:, :], retT[:, mt, qsl],
                                        retT[:, mt, qsl], mul_op)
                bal.note_pool(QCH, eff=0.42)
                nc.vector.tensor_reduce(
                    statT[:, 8 + sidx:9 + sidx], sqs[:, :],
                    axis=mybir.AxisListType.X, op=add_op)
                bal.note_dve(QCH)

            pend_stats = None
            for mt in range(2):
                for c in range(NQC):
                    qsl = slice(c * QCH, (c + 1) * QCH)
                    rp = psacc.tile([P, QCH], f32, tag="acc")
                    # diagonal strip: items (h, o), scores 3 ahead of V;
                    # the first two scores are emitted before the applies so
                    # their copies overlap the apply matmuls.
                    items = [(h, o) for o in range(KPC) for h in range(2)]
                    sstiles = {}

                    def emit_applies():
                        if c == 0:
                            # zero the full bank; everything accumulates
                            nc.tensor.matmul(
                                rp[:, :], lhsT=zmT[:, :],
                                rhs=qhT[:, mt, qsl],
                                start=True, stop=False,
                                skip_group_check=True)
                        for cp in range(c):
                            nc.tensor.matmul(
                                rp[:, :], lhsT=mTile[:, mt, cp, :],
                                rhs=qhT[:, mt, qsl],
                                start=(cp == 0), stop=False,
                                skip_group_check=True)

                    # score widths: o=0,1,3 keep only the 128-wide masked
                    # diagonal square (their strictly-lower columns flow
                    # through the chunk-local M128 states); o=2 also carries
                    # its 128 lower columns explicitly.
                    def swidth(o):
                        return 256 if o == 2 else KBLK

                    def emit_score(h, o):
                        K = KPC * c + o
                        w = swidth(o)
                        prow = slice(64 * h, 64 * (h + 1))
                        sp = ps.tile([P, QCH], f32, tag="mm")
                        nc.tensor.matmul(
                            sp[:, :w], lhsT=khT[prow, mt,
                                               K * KBLK:(K + 1) * KBLK],
                            rhs=qhT[prow, mt,
                                    c * QCH + KBLK * o:
                                    c * QCH + KBLK * o + w],
                            start=True, stop=True)
                        ss = scp.tile([P, QCH], bf16, tag="sc")
                        bal.tt(ss[:, :KBLK], sp[:, :KBLK], triT[:, :],
                               mul_op)
                        if o == 2:
                            bal.copy(ss[:, KBLK:w], sp[:, KBLK:w])
                        return ss

                    def emit_v(h, o, ss):
                        K = KPC * c + o
                        w = swidth(o)
                        vstart = False
                        vstop = (o == KPC - 1)
                        nc.tensor.matmul(
                            rp[64 * h:64 * (h + 1),
                               KBLK * o:KBLK * o + w],
                            lhsT=vT[:, K, mt, 64 * h:64 * (h + 1)],
                            rhs=ss[:, :w],
                            start=vstart, stop=vstop,
                            skip_group_check=True)

                    def emit_m128_build(slot):
                        K = KPC * c + slot
                        mps = pssm.tile([P, P], f32, tag="mps")
                        nc.tensor.matmul(
                            mps[:, :], lhsT=ktT[(mt, c)][:, slot, :],
                            rhs=vT[:, K, mt, :], start=True, stop=True)
                        for hh in range(2):
                            sl = slice(64 * hh, 64 * (hh + 1))
                            bal.copy(m128T[sl, slot, sl], mps[sl, sl])

                    def emit_m128_apply(slot):
                        cols = slice(c * QCH + KBLK * (slot + 1),
                                     (c + 1) * QCH)
                        rcols = slice(KBLK * (slot + 1), QCH)
                        nc.tensor.matmul(
                            rp[:, rcols], lhsT=m128T[:, slot, :],
                            rhs=qhT[:, mt, cols],
                            start=False, stop=False,
                            skip_group_check=True)

                    bidx = mt * NQC + c
                    for idx, (h, o) in enumerate(items):
                        sstiles[idx] = emit_score(h, o)
                        if idx == 0:
                            emit_m128_build(0)
                        if idx == 1:
                            emit_applies()
                        if idx == 2:
                            emit_m128_build(1)
                        if idx == 3 and bidx < len(mjobs):
                            emit_mbuild(*mjobs[bidx])
                        if idx == 4:
                            emit_m128_apply(0)
                            emit_m128_apply(1)
                        if idx >= 3:
                            emit_v(*items[idx - 3], sstiles.pop(idx - 3))
                    for idx in (len(items) - 3, len(items) - 2,
                                len(items) - 1):
                        emit_v(*items[idx], sstiles.pop(idx))
                    # previous chunk's stats go here: the spill waits on
                    # nothing by now and blocks no copies this chunk needed
                    if pend_stats is not None:
                        emit_stats(*pend_stats)
                    pend_stats = (rp, mt, c)
            emit_stats(*pend_stats)

            # ==== Phase C: gate projection with GN finalize tucked inside ====
            # The GN reduction chain (including the tiny per-head-totals PE
            # matmul) is emitted after the second gate chunk so it completes
            # while the remaining gate chunks keep the PE busy; nrm/gtd run
            # q-chunk-major so the output projection never waits.
            gchunks = [(mt, c) for c in range(NQC) for mt in range(2)]

            def emit_gate(mt, c):
                pst = emit_proj("wg", mt, c)
                nc.scalar.activation(
                    gateT[:, mt, c * QCH:(c + 1) * QCH],
                    pst[:, :], silu,
                    bias=biasT["bgr"][:, mt:mt + 1], scale=1.0)
                bal.note_act(QCH)

            def emit_nrm_gtd(c):
                for mt in range(2):
                    csl = slice(c * QCH, (c + 1) * QCH)
                    nrm = nrmp.tile([P, QCH], bf16, tag="nrm")
                    bal.ts(nrm[:, :], retT[:, mt, csl],
                           aff_a[:, mt:mt + 1], aff_b[:, mt:mt + 1],
                           mul_op, add_op)
                    bal.tt(gtdT[:, mt, csl], nrm[:, :], gateT[:, mt, csl],
                           mul_op)

            def emit_outproj(c):
                # bf16 partials; host sums across cores and adds bo
                for tt in range(KPC * c, KPC * (c + 1)):
                    for oc in range(2):
                        op_ps = psacc.tile([P, QCH], f32, tag="acc")
                        for kc in range(2):
                            nc.tensor.matmul(
                                op_ps[:, :],
                                lhsT=gtdT[:, kc, tt * P:(tt + 1) * P],
                                rhs=woT[:, kc, oc * QCH:(oc + 1) * QCH],
                                start=(kc == 0), stop=(kc == 1))
                        ob_t = outp.tile([P, QCH], bf16, tag="ob")
                        bal.copy(ob_t[:, :], op_ps[:, :])
                        nc.sync.dma_start(
                            out_d[tt * P:(tt + 1) * P,
                                  oc * QCH:(oc + 1) * QCH],
                            ob_t[:, :])

            emit_gate(*gchunks[0])
            # GN part 1 on DVE (starts as soon as stats land)
            s1 = finp.tile([P, 4], f32, tag="s1")
            nc.vector.tensor_reduce(
                s1[:, :], statT[:, :].rearrange("p (g c) -> p g c", c=NQC),
                axis=mybir.AxisListType.X, op=add_op)
            s1b = finp.tile([P, 4], bf16, tag="s1b")
            nc.vector.tensor_copy(s1b[:, :], s1[:, :])
            emit_gate(*gchunks[1])
            totp = pssm.tile([P, 4], f32, tag="tot")
            nc.tensor.matmul(totp[:, :], lhsT=obT[:, :], rhs=s1b[:, :],
                             start=True, stop=True)
            tot = finp.tile([P, 4], f32, tag="tot_sb")
            nc.vector.tensor_copy(tot[:, :], totp[:, :])
            mean = finp.tile([P, 2], f32, tag="mean")
            nc.vector.tensor_scalar_mul(mean[:, :], tot[:, 0:2], 1.0 / NELEM)
            ex2 = finp.tile([P, 2], f32, tag="ex2")
            nc.vector.tensor_scalar_mul(ex2[:, :], tot[:, 2:4], 1.0 / NELEM)
            msq = finp.tile([P, 2], f32, tag="msq")
            nc.vector.tensor_tensor(msq[:, :], mean[:, :], mean[:, :], mul_op)
            var = finp.tile([P, 2], f32, tag="var")
            nc.vector.tensor_tensor(var[:, :], ex2[:, :], msq[:, :], sub_op)
            std = finp.tile([P, 2], f32, tag="std")
            nc.scalar.activation(std[:, :], var[:, :], fsqrt,
                                 bias=epsT[:, :], scale=1.0)
            istd = finp.tile([P, 2], f32, tag="istd")
            nc.vector.reciprocal(istd[:, :], std[:, :])
            aff_a = finp.tile([P, 2], f32, tag="aff_a")
            nc.vector.tensor_tensor(aff_a[:, :], istd[:, :], gnwT[:, :], mul_op)
            ma = finp.tile([P, 2], f32, tag="ma")
            nc.vector.tensor_tensor(ma[:, :], mean[:, :], aff_a[:, :], mul_op)
            aff_b = finp.tile([P, 2], f32, tag="aff_b")
            nc.vector.tensor_tensor(aff_b[:, :], gnbT[:, :], ma[:, :], sub_op)
            # remaining gate chunks with nrm/gtd + out-proj groups woven in,
            # so the PE rolls from gate matmuls straight into the output
            # projection with no barrier
            emit_gate(*gchunks[2])
            emit_gate(*gchunks[3])
            emit_nrm_gtd(0)
            emit_gate(*gchunks[4])
            emit_gate(*gchunks[5])
            emit_nrm_gtd(1)
            emit_outproj(0)
            emit_gate(*gchunks[6])
            emit_gate(*gchunks[7])
            emit_nrm_gtd(2)
            emit_outproj(1)
            emit_nrm_gtd(3)
            emit_outproj(2)
            emit_outproj(3)

    nc.compile()
    return nc


_PROGRAM_CACHE = {}


def _get_program(has_bv):
    if has_bv not in _PROGRAM_CACHE:
        _PROGRAM_CACHE[has_bv] = _build_program(has_bv)
    return _PROGRAM_CACHE[has_bv]


def kernel(**inputs):
    global LAST_EXEC_NS, LAST_RESULTS
    in_maps, has_bv = _host_prep(inputs)
    nc = _get_program(has_bv)
    trace = bool(int(os.environ.get("KERNEL_TRACE", "0")))
    kw = {}
    if trace:
        kw["trace"] = True
        kw["trace_cores"] = [int(c) for c in
                             os.environ.get("KERNEL_TRACE_CORES", "0").split(",")]
        td = os.environ.get("KERNEL_TRACE_DIR")
        if td:
            os.makedirs(td, exist_ok=True)
            kw["tmpdir"] = td
    res = run_bass_kernel_spmd(nc, in_maps, list(range(N_CORES)), **kw)
    LAST_EXEC_NS = res.exec_time_ns
    LAST_RESULTS = res
    bo = np.asarray(inputs["bo"], np.float32)
    out = np.zeros((B, S, D), np.float32)
    for core in range(N_CORES):
        out[core // HG] += np.asarray(res.results[core]["out"], np.float32)
    out += bo[None, None, :]
    return out
